# revision 29
# baseline (speedup 1.0000x reference)
"""MoE (8 experts, top-2) Trainium2 Bass kernel, expert-parallel over 8 cores.

Pipeline (all FLOPs on device):
  gate   : gating logits + softmax for all tokens (data-parallel over cores),
           output token-major probs [N, E]
  host   : top-2 selection + slot packing (comparisons/indexing only)
  mlp    : per-core fused 3-layer expert MLP on gathered tokens (bf16 matmuls,
           h1/h2 resident in SBUF, per-token gate weight folded into L3 evict)
  combine: per-token gather of its two scaled expert rows + add
"""

import numpy as np
import ml_dtypes

import jax

jax.config.update("jax_compilation_cache_dir", "/tmp/jax_comp_cache")
jax.config.update("jax_persistent_cache_min_entry_size_bytes", -1)
jax.config.update("jax_persistent_cache_min_compile_time_secs", 0)

import concourse.bass as bass
import concourse.mybir as mybir
import concourse.tile as tile
from concourse import bacc
from concourse.bass_utils import run_bass_kernel_spmd

N, D, H, O, E = 8192, 1024, 2048, 1024, 8
NCORES = 8
TPC = N // NCORES  # tokens per core for gate/combine phases
F32 = mybir.dt.float32
BF16 = mybir.dt.bfloat16
I32 = mybir.dt.int32
BF = ml_dtypes.bfloat16
ACT = mybir.ActivationFunctionType

_CACHE = {}


def _nc():
    return bacc.Bacc(None, target_bir_lowering=False, debug=True)


def _pmn(a, dtype=BF):
    """[K, M] row-major -> [128, K/128, M] with row k = m*128 + p."""
    K, M = a.shape
    return np.ascontiguousarray(
        a.reshape(K // 128, 128, M).transpose(1, 0, 2).astype(dtype)
    )


# ---------------------------------------------------------------- gate
def _build_gate_nc():
    # Logits need fp32-class accuracy so top-2 selection and combine weights
    # match the fp32 reference on near-tied gates (bf16 logits flip ~0.6% of
    # tokens' second expert -> 5e-2 output error). L1 uses a 3-term bf16
    # residual split (err ~2^-17), L2 runs in true fp32 (4 cyc/row, tiny).
    nc = _nc()
    NT = TPC // 128  # token tiles per core
    xhi = nc.dram_tensor("xhi", [128, D // 128, TPC], BF16, kind="ExternalInput")
    xlo = nc.dram_tensor("xlo", [128, D // 128, TPC], BF16, kind="ExternalInput")
    wg1hi = nc.dram_tensor("wg1hi", [128, D // 128, 128], BF16, kind="ExternalInput")
    wg1lo = nc.dram_tensor("wg1lo", [128, D // 128, 128], BF16, kind="ExternalInput")
    wg2 = nc.dram_tensor("wg2", [128, 128], F32, kind="ExternalInput")
    probs = nc.dram_tensor("probs", [128, NT, E], F32, kind="ExternalOutput")
    X = mybir.AxisListType.X
    with tile.TileContext(nc) as tc:
        with (
            tc.tile_pool(name="w", bufs=1) as wp,
            tc.tile_pool(name="x", bufs=2) as xp,
            tc.tile_pool(name="g", bufs=1) as gp,
            tc.tile_pool(name="s", bufs=2) as sp,
            tc.tile_pool(name="ps", bufs=3, space="PSUM") as pp,
            tc.tile_pool(name="warm", bufs=1, space="PSUM") as wmp,
        ):
            whi = wp.tile([128, D // 128, 128], BF16)
            wlo = wp.tile([128, D // 128, 128], BF16)
            wg2t = wp.tile([128, 128], F32)
            nc.sync.dma_start(whi[:], wg1hi[:])
            nc.sync.dma_start(wlo[:], wg1lo[:])
            nc.sync.dma_start(wg2t[:], wg2[:])
            # PE p-state warmup: the cost model ramps 0.65->1.2->2.4 GHz over
            # 3us of continuous PE activity; dummy matmuls on the (early)
            # weight tile keep the clock ramping while x streams in.
            warm = wmp.tile([128, 128], F32, tag="warm")
            for _ in range(44):
                nc.tensor.matmul(warm[:], lhsT=whi[:, 0], rhs=whi[:, 1], start=True, stop=True)
            # L1: g1 = relu(Wg1^T x) ~= relu((Whi+Wlo)^T xhi + Whi^T xlo)
            # x loaded per 512-token chunk so DMA pipelines with compute
            g1 = gp.tile([128, NT // 4, 512], F32)  # [feat, chunk, token]
            for ci, i in enumerate(range(0, TPC, 512)):
                xh = xp.tile([128, D // 128, 512], BF16, tag="xh", name="xh")
                xl = xp.tile([128, D // 128, 512], BF16, tag="xl", name="xl")
                nc.sync.dma_start(xh[:], xhi[:, :, i : i + 512])
                nc.sync.dma_start(xl[:], xlo[:, :, i : i + 512])
                ps = pp.tile([128, 512], F32, tag="ps1")
                terms = [(whi, xh), (wlo, xh), (whi, xl)]
                for ti, (wt, xt_) in enumerate(terms):
                    for k in range(D // 128):
                        nc.tensor.matmul(
                            ps[:],
                            lhsT=wt[:, k],
                            rhs=xt_[:, k],
                            start=(ti == 0 and k == 0),
                            stop=(ti == 2 and k == D // 128 - 1),
                        )
                nc.scalar.activation(g1[:, ci], ps[:], ACT.Relu)
            # L2 (token-major, fp32): logitsT[t, e] = sum_f g1[f, t] Wg2[f, e]
            ex = sp.tile([128, NT, E], F32, tag="ex")
            for i in range(NT):
                ps = pp.tile([128, 128], F32, tag="ps2")
                nc.tensor.matmul(
                    ps[:],
                    lhsT=g1[:, i // 4, (i % 4) * 128 : (i % 4 + 1) * 128],
                    rhs=wg2t[:],
                    start=True,
                    stop=True,
                )
                nc.scalar.activation(ex[:, i], ps[:, :E], ACT.Exp)
            s = sp.tile([128, NT, 1], F32, tag="s")
            nc.vector.reduce_sum(out=s[:], in_=ex[:], axis=X)
            inv = sp.tile([128, NT, 1], F32, tag="inv")
            nc.vector.reciprocal(out=inv[:], in_=s[:])
            pr = sp.tile([128, NT, E], F32, tag="pr")
            nc.vector.tensor_mul(out=pr[:], in0=ex[:], in1=inv[:].to_broadcast((128, NT, E)))
            nc.sync.dma_start(probs[:], pr[:])
    nc.compile()
    return nc


# ---------------------------------------------------------------- mlp
def _build_mlp_nc(tpl):
    """Fused 3-layer expert MLP; one slot per template entry, bf16 matmuls.

    Slot j: xt{j} [128, D/128, S] bf16, w1_{j} [128, D/128, H], w2_{j}
    [128, H/128, H], w3_{j} [128, H/128, O] bf16, wv{j} [128, S] f32
    (per-token combine weight, pre-broadcast), output yt{j} [128, O/128, S]
    bf16 (already scaled by wv).
    """
    nc = _nc()
    t = {}
    for j, S in enumerate(tpl):
        # weights pre-tiled host-side: [m][p, k, col] so each m-tile DMA is
        # one contiguous 2KB-per-partition transfer (no small-descriptor
        # penalty)
        t[f"xt{j}"] = nc.dram_tensor(f"xt{j}", [128, D // 128, S], BF16, kind="ExternalInput")
        t[f"w1_{j}"] = nc.dram_tensor(f"w1_{j}", [H // 128, 128, D // 128, 128], BF16, kind="ExternalInput")
        t[f"w2_{j}"] = nc.dram_tensor(f"w2_{j}", [H // 128, 128, H // 128, 128], BF16, kind="ExternalInput")
        t[f"w3_{j}"] = nc.dram_tensor(f"w3_{j}", [O // 128, 128, H // 128, 128], BF16, kind="ExternalInput")
        t[f"wv{j}"] = nc.dram_tensor(f"wv{j}", [128, S], F32, kind="ExternalInput")
        t[f"yt{j}"] = nc.dram_tensor(f"yt{j}", [128, O // 128, S], BF16, kind="ExternalOutput")
    Smax = max(tpl)
    with tile.TileContext(nc) as tc:
        with (
            tc.tile_pool(name="x", bufs=2) as xp,
            tc.tile_pool(name="w1", bufs=3) as w1p,
            tc.tile_pool(name="w2", bufs=3) as w2p,
            tc.tile_pool(name="w3", bufs=3) as w3p,
            tc.tile_pool(name="h", bufs=2) as hp,
            tc.tile_pool(name="wv", bufs=2) as wvp,
            tc.tile_pool(name="y", bufs=4) as yp,
            tc.tile_pool(name="ps", bufs=6, space="PSUM") as pp,
        ):
            for j, S in enumerate(tpl):
                chunks = [(o, min(512, S - o)) for o in range(0, S, 512)]
                xsb = xp.tile([128, D // 128, Smax], BF16, tag="x")
                for k in range(D // 128):
                    nc.sync.dma_start(xsb[:, k, :S], t[f"xt{j}"][:, k])
                wv = wvp.tile([128, Smax], F32, tag="wv")
                nc.sync.dma_start(wv[:, :S], t[f"wv{j}"][:])
                h1 = hp.tile([128, H // 128, Smax], BF16, tag="h1")
                for m in range(H // 128):
                    w = w1p.tile([128, D // 128, 128], BF16, tag="w1")
                    nc.sync.dma_start(w[:], t[f"w1_{j}"][m])
                    for o, sz in chunks:
                        ps = pp.tile([128, 512], F32, tag="ps", name="ps")[:, :sz]
                        for k in range(D // 128):
                            nc.tensor.matmul(
                                ps,
                                lhsT=w[:, k],
                                rhs=xsb[:, k, o : o + sz],
                                start=(k == 0),
                                stop=(k == D // 128 - 1),
                            )
                        nc.scalar.activation(h1[:, m, o : o + sz], ps, ACT.Relu)
                h2 = hp.tile([128, H // 128, Smax], BF16, tag="h2")
                for m in range(H // 128):
                    w = w2p.tile([128, H // 128, 128], BF16, tag="w2")
                    nc.sync.dma_start(w[:], t[f"w2_{j}"][m])
                    for o, sz in chunks:
                        ps = pp.tile([128, 512], F32, tag="ps", name="ps")[:, :sz]
                        for k in range(H // 128):
                            nc.tensor.matmul(
                                ps,
                                lhsT=w[:, k],
                                rhs=h1[:, k, o : o + sz],
                                start=(k == 0),
                                stop=(k == H // 128 - 1),
                            )
                        nc.scalar.activation(h2[:, m, o : o + sz], ps, ACT.Relu)
                for m in range(O // 128):
                    w = w3p.tile([128, H // 128, 128], BF16, tag="w3")
                    nc.sync.dma_start(w[:], t[f"w3_{j}"][m])
                    for o, sz in chunks:
                        ps = pp.tile([128, 512], F32, tag="ps", name="ps")[:, :sz]
                        for k in range(H // 128):
                            nc.tensor.matmul(
                                ps,
                                lhsT=w[:, k],
                                rhs=h2[:, k, o : o + sz],
                                start=(k == 0),
                                stop=(k == H // 128 - 1),
                            )
                        y = yp.tile([128, 512], BF16, tag="y", name="y")[:, :sz]
                        nc.vector.tensor_mul(out=y, in0=ps, in1=wv[:, o : o + sz])
                        nc.sync.dma_start(t[f"yt{j}"][:, m, o : o + sz], y)
    nc.compile()
    return nc


# ---------------------------------------------------------------- fp8 mlp
# 3-term residual-split fp8 matmuls in DoubleRow perf mode (2 contraction
# rows per PE pass): W ~= (Whi + Wlo)/sw, x ~= (xhi + xlo)/sx, and
# W^T x ~= (Whi^T xhi + Wlo^T xhi + Whi^T xlo) / (sw sx), dropping only the
# lo*lo term (~1e-3 relative). Measured end-to-end rel err ~3.4e-3.
FP8 = mybir.dt.float8e4
E4M3 = ml_dtypes.float8_e4m3
SX, SH = 32.0, 32.0
SW1, SW2, SW3 = 512.0, 1024.0, 1024.0
USE_FP8 = True


def _build_mlp_fp8_nc(tpl):
    nc = _nc()
    t = {}
    KD, KH = D // 128, H // 128
    for j, S in enumerate(tpl):
        t[f"xhi{j}"] = nc.dram_tensor(f"xhi{j}", [128, KD, S], FP8, kind="ExternalInput")
        t[f"xlo{j}"] = nc.dram_tensor(f"xlo{j}", [128, KD, S], FP8, kind="ExternalInput")
        for L, KT, MT in (("1", KD, KH), ("2", KH, KH), ("3", KH, O // 128)):
            # hi and lo halves concatenated along k so one DMA loads both
            # (DMA issue costs ~650ns of SEQ+HWDGE each; count matters)
            t[f"w{L}_{j}"] = nc.dram_tensor(
                f"w{L}_{j}", [MT, 128, 2 * KT, 128], FP8, kind="ExternalInput"
            )
        t[f"wv{j}"] = nc.dram_tensor(f"wv{j}", [128, S], F32, kind="ExternalInput")
        t[f"yt{j}"] = nc.dram_tensor(f"yt{j}", [128, O // 128, S], BF16, kind="ExternalOutput")
    Smax = max(tpl)
    DR = mybir.MatmulPerfMode.DoubleRow

    with tile.TileContext(nc) as tc:
        with (
            tc.tile_pool(name="x", bufs=2) as xp,
            tc.tile_pool(name="w1", bufs=4) as w1p,
            tc.tile_pool(name="w2", bufs=6) as w2p,
            tc.tile_pool(name="w3", bufs=4) as w3p,
            tc.tile_pool(name="h", bufs=1) as hp,
            tc.tile_pool(name="hb", bufs=4) as hbp,
            tc.tile_pool(name="wv", bufs=2) as wvp,
            tc.tile_pool(name="y", bufs=3) as yp,
            tc.tile_pool(name="ps", bufs=6, space="PSUM") as pp,
            tc.tile_pool(name="warm", bufs=1, space="PSUM") as wmp,
        ):
            # PE p-state warmup from a memset tile (no DMA dependency):
            # dummies start ~0.5us in, so the clock is at 2.4GHz before the
            # first real matmul's operands land.
            wsrc = hbp.tile([128, 128], BF16, tag="wsrc", name="wsrc")
            nc.vector.memset(wsrc[:], 0.0)
            warm = wmp.tile([128, 128], F32, tag="warm")
            for _ in range(32):
                nc.tensor.matmul(warm[:], lhsT=wsrc[:], rhs=wsrc[:], start=True, stop=True)
            for j, S in enumerate(tpl):
                chunks = [(o, min(512, S - o)) for o in range(0, S, 512)]
                # split x loads so the first matmul group's operands (k-tiles
                # 0..1 of xhi then xlo) land first; wv is deferred to L3
                xh = xp.tile([128, KD, Smax], FP8, tag="xh")
                xl = xp.tile([128, KD, Smax], FP8, tag="xl")
                nc.sync.dma_start(xh[:, :2, :S], t[f"xhi{j}"][:, :2])
                nc.sync.dma_start(xh[:, 2:, :S], t[f"xhi{j}"][:, 2:])
                nc.sync.dma_start(xl[:, :2, :S], t[f"xlo{j}"][:, :2])
                nc.sync.dma_start(xl[:, 2:, :S], t[f"xlo{j}"][:, 2:])
                wv = wvp.tile([128, Smax], F32, tag="wv")

                def layer(L, KT, MT, ihi, ilo, wpool, out_cb):
                    for m in range(MT):
                        w = wpool.tile([128, 2 * KT, 128], FP8, tag=f"w{L}")
                        nc.sync.dma_start(w[:], t[f"w{L}_{j}"][m])
                        whi, wlo = w[:, :KT], w[:, KT:]
                        terms = [(whi, ihi), (wlo, ihi), (whi, ilo)]
                        for o, sz in chunks:
                            ps = pp.tile([128, 512], F32, tag="ps", name="ps")[:, :sz]
                            for ti, (wt, it) in enumerate(terms):
                                for k2 in range(KT // 2):
                                    nc.tensor.matmul(
                                        ps,
                                        lhsT=wt[:, 2 * k2 : 2 * k2 + 2, :],
                                        rhs=it[:, 2 * k2 : 2 * k2 + 2, o : o + sz],
                                        start=(ti == 0 and k2 == 0),
                                        stop=(ti == 2 and k2 == KT // 2 - 1),
                                        perf_mode=DR,
                                    )
                            out_cb(m, o, sz, ps)

                h1hi = hp.tile([128, KH, Smax], FP8, tag="h1hi")
                h1lo = hp.tile([128, KH, Smax], FP8, tag="h1lo")

                def evict1(m, o, sz, ps):
                    hb = hbp.tile([128, 512], BF16, tag="hb", name="hb")[:, :sz]
                    nc.scalar.activation(hb, ps, ACT.Relu, scale=SH / (SW1 * SX))
                    nc.scalar.activation(h1hi[:, m, o : o + sz], hb, ACT.Copy)
                    nc.vector.tensor_tensor(
                        h1lo[:, m, o : o + sz], hb, h1hi[:, m, o : o + sz],
                        mybir.AluOpType.subtract,
                    )

                layer("1", KD, KH, xh, xl, w1p, evict1)

                h2hi = hp.tile([128, KH, Smax], FP8, tag="h2hi")
                h2lo = hp.tile([128, KH, Smax], FP8, tag="h2lo")

                def evict2(m, o, sz, ps):
                    hb = hbp.tile([128, 512], BF16, tag="hb", name="hb")[:, :sz]
                    nc.scalar.activation(hb, ps, ACT.Relu, scale=1.0 / SW2)
                    nc.scalar.activation(h2hi[:, m, o : o + sz], hb, ACT.Copy)
                    nc.vector.tensor_tensor(
                        h2lo[:, m, o : o + sz], hb, h2hi[:, m, o : o + sz],
                        mybir.AluOpType.subtract,
                    )

                layer("2", KH, KH, h1hi, h1lo, w2p, evict2)

                nc.sync.dma_start(wv[:, :S], t[f"wv{j}"][:])
                ycur = [None]

                def evict3(m, o, sz, ps):
                    if ycur[0] is None:
                        ycur[0] = yp.tile([128, Smax], BF16, tag="y", name="y")
                    nc.vector.tensor_mul(
                        out=ycur[0][:, o : o + sz], in0=ps, in1=wv[:, o : o + sz]
                    )
                    if o + sz >= S:  # last chunk of this m: flush one DMA
                        nc.sync.dma_start(t[f"yt{j}"][:, m, :S], ycur[0][:, :S])
                        ycur[0] = None

                layer("3", KH, O // 128, h2hi, h2lo, w3p, evict3)
    nc.compile()
    return nc


# ---------------------------------------------------------------- combine
def _build_comb_nc(R):
    nc = _nc()
    NT = TPC // 128
    yall = nc.dram_tensor("yall", [R, O], BF16, kind="ExternalInput")
    i0 = nc.dram_tensor("i0", [128, NT], I32, kind="ExternalInput")
    i1 = nc.dram_tensor("i1", [128, NT], I32, kind="ExternalInput")
    out = nc.dram_tensor("out", [128, NT, O], BF16, kind="ExternalOutput")
    with tile.TileContext(nc) as tc:
        with (
            tc.tile_pool(name="big", bufs=4) as gp,
            tc.tile_pool(name="idx", bufs=1) as ip,
        ):
            i0_t = ip.tile([128, NT], I32)
            i1_t = ip.tile([128, NT], I32)
            nc.sync.dma_start(i0_t[:], i0[:])
            nc.sync.dma_start(i1_t[:], i1[:])
            # per-tile single-index gathers (the runtime only supports one
            # index per partition per indirect DMA). Independent g0/g1 tiles
            # keep the Pool SWDGE queue saturated; DVE adds and per-tile
            # output writes pipeline underneath.
            for i in range(NT):
                g0 = gp.tile([128, O], BF16, tag="g0", name="g0")
                g1 = gp.tile([128, O], BF16, tag="g1", name="g1")
                nc.gpsimd.indirect_dma_start(
                    out=g0[:],
                    out_offset=None,
                    in_=yall[:],
                    in_offset=bass.IndirectOffsetOnAxis(ap=i0_t[:, i : i + 1], axis=0),
                )
                nc.gpsimd.indirect_dma_start(
                    out=g1[:],
                    out_offset=None,
                    in_=yall[:],
                    in_offset=bass.IndirectOffsetOnAxis(ap=i1_t[:, i : i + 1], axis=0),
                )
                nc.vector.tensor_add(out=g0[:], in0=g0[:], in1=g1[:])
                nc.sync.dma_start(out[:, i], g0[:])
    nc.compile()
    return nc


# ---------------------------------------------------------------- packing
def _try_assign(units, tpl_u, slack_cap):
    """Cover each expert's unit count with slots from 8x tpl_u inventory.

    units/tpl_u are in 128-token units. slack_cap bounds per-expert
    over-allocation (in units). Returns {expert: {size: n}} or None.
    """
    inv = {}
    for s in tpl_u:
        inv[s] = inv.get(s, 0) + NCORES
    sizes = sorted(inv, reverse=True)
    experts = sorted(range(len(units)), key=lambda e: -units[e])
    budget = [0]

    def covers(c, i, slack, out, cur):
        if len(out) >= 40 or budget[0] > 100000:
            return
        budget[0] += 1
        if c <= 0:
            if -c <= slack:
                out.append(dict(cur))
            return
        if i >= len(sizes):
            return
        s = sizes[i]
        hi = min(inv[s], (c + slack) // s)
        for n in range(hi, -1, -1):
            if i == len(sizes) - 1 and n * s < c:
                break  # last size can't cover the remainder
            if n:
                cur[s] = n
            covers(c - n * s, i + 1, slack, out, cur)
            cur.pop(s, None)

    def dfs(idx):
        budget[0] += 1
        if budget[0] > 100000:
            return None
        if idx == len(experts):
            return {}
        e = experts[idx]
        c = int(units[e])
        if c == 0:
            rest = dfs(idx + 1)
            if rest is not None:
                rest[e] = {}
            return rest
        out = []
        covers(c, 0, slack_cap, out, {})
        out.sort(key=lambda d: (sum(s * n for s, n in d.items()) - c, sum(d.values())))
        for cov in out[:16]:
            if not all(inv[s] >= n for s, n in cov.items()):
                continue
            for s, n in cov.items():
                inv[s] -= n
            rest = dfs(idx + 1)
            if rest is not None:
                rest[e] = cov
                return rest
            for s, n in cov.items():
                inv[s] += n
        return None

    return dfs(0)


def _solve_packing(counts):
    """Pick per-core slot-size template (uniform across cores) + expert cover.

    Works in 128-token units; returns (template_in_tokens, assign) where
    assign maps expert -> {slot_size_tokens: n_slots}.
    """
    G = 32  # slot-size granularity in tokens
    units = [-(-int(c) // G) for c in counts]
    U0 = max(1, -(-sum(units) // NCORES))
    maxu = max(units + [1])

    def partitions(total, maxpart, maxparts):
        if total == 0:
            yield ()
            return
        if maxparts == 0:
            return
        for p in range(min(total, maxpart), 0, -1):
            for rest in partitions(total - p, p, maxparts - 1):
                yield (p,) + rest

    for U in range(U0, U0 + 24):
        tpls = sorted(
            {t for t in partitions(U, 1024 // G, 5)},
            key=lambda t: (len(t), -min(t), tuple(-v for v in t)),
        )[:64]
        for slack in (0, 1, 2, 4):
            for tpl_u in tpls:
                asg = _try_assign(units, tpl_u, slack)
                if asg is not None:
                    tpl = tuple(s * G for s in tpl_u)
                    return tpl, {
                        e: {s * G: n for s, n in cov.items()}
                        for e, cov in asg.items()
                    }
    # retry coarser granularity before the uniform fallback
    for G2 in (64, 128):
        U0b = max(1, -(-sum(-(-int(c) // G2) for c in counts) // NCORES))
        for U in range(U0b, U0b + 24):
            tpls = sorted(
                {t for t in partitions(U, 1024 // G2, 5)},
                key=lambda t: (len(t), -min(t), tuple(-v for v in t)),
            )[:64]
            for slack in (0, 1, 2, 4):
                for tpl_u in tpls:
                    asg = _try_assign([-(-int(c) // G2) for c in counts], tpl_u, slack)
                    if asg is not None:
                        return tuple(s * G2 for s in tpl_u), {
                            e: {s * G2: n for s, n in cov.items()}
                            for e, cov in asg.items()
                        }
    # ultimate fallback: uniform 2 slots covering the largest expert
    S = max(128, (int(max(counts)) // 2 // 128 + 1) * 128)
    while True:
        if sum(-(-int(c) // S) for c in counts if c) <= 2 * NCORES:
            asg = {e: {S: -(-int(c) // S)} for e, c in enumerate(counts) if c}
            return (S, S), asg
        S += 128
    raise RuntimeError("no packing found")


# ---------------------------------------------------------------- kernel
def kernel(x, W1, b1, W2, b2, W3, b3, Wg1, bg1, Wg2, bg2, top_k):
    x = np.asarray(x, np.float32)
    W1 = np.asarray(W1, np.float32)
    W2 = np.asarray(W2, np.float32)
    W3 = np.asarray(W3, np.float32)
    Wg1 = np.asarray(Wg1, np.float32)
    Wg2 = np.asarray(Wg2, np.float32)
    assert int(np.asarray(top_k)) == 2
    for b in (b1, b2, b3, bg1, bg2):
        assert not np.any(np.asarray(b)), "nonzero biases unsupported"

    core_ids = list(range(NCORES))
    NT = TPC // 128

    # ---------------- gate: logits + softmax on device ----------------
    if "gate" not in _CACHE:
        _CACHE["gate"] = _build_gate_nc()
    xT32 = np.ascontiguousarray(x.T)  # [D, N] fp32
    xT = xT32.astype(BF)  # bf16 hi part, also the expert-MLP input
    xTlo = (xT32 - xT.astype(np.float32)).astype(BF)  # bf16 residual
    wg1p = np.zeros((D, 128), np.float32)
    wg1p[:, :64] = Wg1
    wg2p = np.zeros((128, 128), np.float32)
    wg2p[:64, :E] = Wg2
    wg1hi = _pmn(wg1p)
    wg1lo = _pmn(wg1p - wg1hi.transpose(1, 0, 2).reshape(D, 128).astype(np.float32))

    def _ptile(a, c):  # [D, N] -> per-core [128, D/128, TPC]
        return np.ascontiguousarray(
            a[:, c * TPC : (c + 1) * TPC].reshape(D // 128, 128, TPC).transpose(1, 0, 2)
        )

    in1 = [
        {
            "xhi": _ptile(xT, c),
            "xlo": _ptile(xTlo, c),
            "wg1hi": wg1hi,
            "wg1lo": wg1lo,
            "wg2": wg2p,
        }
        for c in core_ids
    ]
    res1 = run_bass_kernel_spmd(_CACHE["gate"], in1, core_ids).results
    # probs[p, i, e] -> token c*TPC + i*128 + p
    probs = np.concatenate(
        [res1[c]["probs"].transpose(1, 0, 2).reshape(TPC, E) for c in core_ids], axis=0
    ).astype(np.float32)  # [N, E]

    # ---------------- host routing (comparisons/indexing only) ----------------
    top2 = np.argsort(-probs, axis=1, kind="stable")[:, :2]  # [N, 2]
    e0s, e1s = top2[:, 0], top2[:, 1]
    expert_lists = [np.nonzero((top2 == e).any(axis=1))[0] for e in range(E)]
    counts = np.array([len(t) for t in expert_lists])

    tpl, asg = _solve_packing(counts)
    C = sum(tpl)

    # slot positions per size: (core, slot_idx) pools
    pos_pool = {}
    for jj, s in enumerate(tpl):
        pos_pool.setdefault(s, []).extend((c, jj) for c in core_ids)
    slot_off = {}  # (core, j) -> global row offset in yall
    for c in core_ids:
        off = c * C
        for jj, s in enumerate(tpl):
            slot_off[(c, jj)] = off
            off += s
    # assign slots to experts, chop token lists
    slot_map = {}  # (core, j) -> (expert, token_array)
    glob_row = np.zeros((N, E), np.int64)
    for e in range(E):
        tl = expert_lists[e]
        claims = []
        for s in sorted(asg.get(e, {}), reverse=True):
            for _ in range(asg[e][s]):
                claims.append((s, pos_pool[s].pop()))
        cum = 0
        for s, posn in claims:
            chunk = tl[cum : cum + s]
            slot_map[posn] = (e, chunk)
            glob_row[chunk, e] = slot_off[posn] + np.arange(len(chunk))
            cum += s
        assert cum >= len(tl), f"expert {e} not fully covered"
    # leftover slots empty
    for s, pool in pos_pool.items():
        for posn in pool:
            slot_map[posn] = (0, np.zeros(0, np.int64))

    w_tok = np.take_along_axis(probs, top2, axis=1)  # [N, 2] combine weights

    # ---------------- mlp: fused expert MLP on device ----------------
    key2 = (("emlp8" if USE_FP8 else "emlp"), tpl)
    if key2 not in _CACHE:
        _CACHE[key2] = _build_mlp_fp8_nc(tpl) if USE_FP8 else _build_mlp_nc(tpl)
    Wb = {}

    def _mtile(a):
        # [128, K/128, M] -> [M/128, 128, K/128, 128] (m-tile contiguous)
        P, KT, M = a.shape
        return np.ascontiguousarray(
            a.reshape(P, KT, M // 128, 128).transpose(2, 0, 1, 3)
        )

    def _split8(W, sw):
        Ws = W * sw
        hi = Ws.astype(E4M3)
        lo = (Ws - hi.astype(np.float32)).astype(E4M3)
        # concat hi and lo along the k-tile axis: [MT, 128, 2*KT, 128]
        return np.ascontiguousarray(
            np.concatenate([_mtile(_pmn(hi, E4M3)), _mtile(_pmn(lo, E4M3))], axis=2)
        )

    def wts(e):
        if e not in Wb:
            if USE_FP8:
                Wb[e] = (
                    _split8(W1[e], SW1),
                    _split8(W2[e], SW2),
                    _split8(W3[e], SW3),
                )
            else:
                Wb[e] = (_mtile(_pmn(W1[e])), _mtile(_pmn(W2[e])), _mtile(_pmn(W3[e])))
        return Wb[e]

    if USE_FP8:
        Xs = xT32 * SX
        xhi8 = Xs.astype(E4M3)
        xlo8 = (Xs - xhi8.astype(np.float32)).astype(E4M3)

    wmask = np.zeros((N, E), np.float32)
    wmask[np.arange(N), e0s] = w_tok[:, 0]
    wmask[np.arange(N), e1s] = w_tok[:, 1]

    def _pt(a, S):  # [D, S] -> [128, D/128, S]
        return np.ascontiguousarray(a.reshape(D // 128, 128, S).transpose(1, 0, 2))

    in2 = []
    for c in core_ids:
        d = {}
        for jj, S in enumerate(tpl):
            e, chunk = slot_map[(c, jj)]
            padded = np.zeros(S, np.int64)
            padded[: len(chunk)] = chunk
            wv = np.zeros(S, np.float32)
            wv[: len(chunk)] = wmask[chunk, e]
            if USE_FP8:
                d[f"xhi{jj}"] = _pt(xhi8[:, padded], S)
                d[f"xlo{jj}"] = _pt(xlo8[:, padded], S)
                d[f"w1_{jj}"], d[f"w2_{jj}"], d[f"w3_{jj}"] = wts(e)
                wv = wv / (SW3 * SH)
            else:
                w1p, w2p, w3p = wts(e)
                d[f"xt{jj}"] = _pt(xT[:, padded], S)
                d[f"w1_{jj}"] = w1p
                d[f"w2_{jj}"] = w2p
                d[f"w3_{jj}"] = w3p
            d[f"wv{jj}"] = np.ascontiguousarray(
                np.broadcast_to(wv[None, :], (128, S))
            )
        in2.append(d)
    res2 = run_bass_kernel_spmd(_CACHE[key2], in2, core_ids).results

    R = NCORES * C
    yall = np.empty((R, O), BF)
    for c in core_ids:
        for jj, S in enumerate(tpl):
            yt = res2[c][f"yt{jj}"]  # [128, O/128, S]
            off = slot_off[(c, jj)]
            yall[off : off + S] = yt.transpose(2, 1, 0).reshape(S, O)

    # ---------------- combine: gather 2 scaled rows + add ----------------
    key3 = ("comb", R)
    if key3 not in _CACHE:
        _CACHE[key3] = _build_comb_nc(R)
    g0 = glob_row[np.arange(N), e0s].astype(np.int32)
    g1 = glob_row[np.arange(N), e1s].astype(np.int32)
    in3 = []
    for c in core_ids:
        sl = slice(c * TPC, (c + 1) * TPC)
        in3.append(
            {
                "yall": yall,
                "i0": np.ascontiguousarray(g0[sl].reshape(NT, 128).T),
                "i1": np.ascontiguousarray(g1[sl].reshape(NT, 128).T),
            }
        )
    res3 = run_bass_kernel_spmd(_CACHE[key3], in3, core_ids).results
    out = np.concatenate(
        [
            res3[c]["out"].transpose(1, 0, 2).reshape(TPC, O).astype(np.float32)
            for c in core_ids
        ],
        axis=0,
    )
    return out


# revision 30
# speedup vs baseline: 1.0118x; 1.0118x over previous
"""MoE (8 experts, top-2) Trainium2 Bass kernel, expert-parallel over 8 cores.

Pipeline (all FLOPs on device):
  gate   : gating logits + softmax for all tokens (data-parallel over cores),
           output token-major probs [N, E]
  host   : top-2 selection + slot packing (comparisons/indexing only)
  mlp    : per-core fused 3-layer expert MLP on gathered tokens (bf16 matmuls,
           h1/h2 resident in SBUF, per-token gate weight folded into L3 evict)
  combine: per-token gather of its two scaled expert rows + add
"""

import numpy as np
import ml_dtypes

import jax

jax.config.update("jax_compilation_cache_dir", "/tmp/jax_comp_cache")
jax.config.update("jax_persistent_cache_min_entry_size_bytes", -1)
jax.config.update("jax_persistent_cache_min_compile_time_secs", 0)

import concourse.bass as bass
import concourse.mybir as mybir
import concourse.tile as tile
from concourse import bacc
from concourse.bass_utils import run_bass_kernel_spmd

N, D, H, O, E = 8192, 1024, 2048, 1024, 8
NCORES = 8
TPC = N // NCORES  # tokens per core for gate/combine phases
F32 = mybir.dt.float32
BF16 = mybir.dt.bfloat16
I32 = mybir.dt.int32
BF = ml_dtypes.bfloat16
ACT = mybir.ActivationFunctionType

_CACHE = {}


def _nc():
    return bacc.Bacc(None, target_bir_lowering=False, debug=True)


def _pmn(a, dtype=BF):
    """[K, M] row-major -> [128, K/128, M] with row k = m*128 + p."""
    K, M = a.shape
    return np.ascontiguousarray(
        a.reshape(K // 128, 128, M).transpose(1, 0, 2).astype(dtype)
    )


# ---------------------------------------------------------------- gate
def _build_gate_nc():
    # Logits need fp32-class accuracy so top-2 selection and combine weights
    # match the fp32 reference on near-tied gates (bf16 logits flip ~0.6% of
    # tokens' second expert -> 5e-2 output error). L1 uses a 3-term bf16
    # residual split (err ~2^-17), L2 runs in true fp32 (4 cyc/row, tiny).
    nc = _nc()
    NT = TPC // 128  # token tiles per core
    xhi = nc.dram_tensor("xhi", [128, D // 128, TPC], BF16, kind="ExternalInput")
    xlo = nc.dram_tensor("xlo", [128, D // 128, TPC], BF16, kind="ExternalInput")
    wg1hi = nc.dram_tensor("wg1hi", [128, D // 128, 128], BF16, kind="ExternalInput")
    wg1lo = nc.dram_tensor("wg1lo", [128, D // 128, 128], BF16, kind="ExternalInput")
    wg2 = nc.dram_tensor("wg2", [128, 128], F32, kind="ExternalInput")
    probs = nc.dram_tensor("probs", [128, NT, E], F32, kind="ExternalOutput")
    X = mybir.AxisListType.X
    with tile.TileContext(nc) as tc:
        with (
            tc.tile_pool(name="w", bufs=1) as wp,
            tc.tile_pool(name="x", bufs=2) as xp,
            tc.tile_pool(name="g", bufs=1) as gp,
            tc.tile_pool(name="s", bufs=2) as sp,
            tc.tile_pool(name="ps", bufs=3, space="PSUM") as pp,
            tc.tile_pool(name="warm", bufs=1, space="PSUM") as wmp,
        ):
            whi = wp.tile([128, D // 128, 128], BF16)
            wlo = wp.tile([128, D // 128, 128], BF16)
            wg2t = wp.tile([128, 128], F32)
            nc.sync.dma_start(whi[:], wg1hi[:])
            nc.sync.dma_start(wlo[:], wg1lo[:])
            nc.sync.dma_start(wg2t[:], wg2[:])
            # PE p-state warmup: the cost model ramps 0.65->1.2->2.4 GHz over
            # 3us of continuous PE activity; dummy matmuls on the (early)
            # weight tile keep the clock ramping while x streams in.
            warm = wmp.tile([128, 128], F32, tag="warm")
            for _ in range(44):
                nc.tensor.matmul(warm[:], lhsT=whi[:, 0], rhs=whi[:, 1], start=True, stop=True)
            # L1: g1 = relu(Wg1^T x) ~= relu((Whi+Wlo)^T xhi + Whi^T xlo)
            # x loaded per 512-token chunk so DMA pipelines with compute
            g1 = gp.tile([128, NT // 4, 512], F32)  # [feat, chunk, token]
            for ci, i in enumerate(range(0, TPC, 512)):
                xh = xp.tile([128, D // 128, 512], BF16, tag="xh", name="xh")
                xl = xp.tile([128, D // 128, 512], BF16, tag="xl", name="xl")
                nc.sync.dma_start(xh[:], xhi[:, :, i : i + 512])
                nc.sync.dma_start(xl[:], xlo[:, :, i : i + 512])
                ps = pp.tile([128, 512], F32, tag="ps1")
                terms = [(whi, xh), (wlo, xh), (whi, xl)]
                for ti, (wt, xt_) in enumerate(terms):
                    for k in range(D // 128):
                        nc.tensor.matmul(
                            ps[:],
                            lhsT=wt[:, k],
                            rhs=xt_[:, k],
                            start=(ti == 0 and k == 0),
                            stop=(ti == 2 and k == D // 128 - 1),
                        )
                nc.scalar.activation(g1[:, ci], ps[:], ACT.Relu)
            # L2 (token-major, fp32): logitsT[t, e] = sum_f g1[f, t] Wg2[f, e]
            ex = sp.tile([128, NT, E], F32, tag="ex")
            for i in range(NT):
                ps = pp.tile([128, 128], F32, tag="ps2")
                nc.tensor.matmul(
                    ps[:],
                    lhsT=g1[:, i // 4, (i % 4) * 128 : (i % 4 + 1) * 128],
                    rhs=wg2t[:],
                    start=True,
                    stop=True,
                )
                nc.scalar.activation(ex[:, i], ps[:, :E], ACT.Exp)
            s = sp.tile([128, NT, 1], F32, tag="s")
            nc.vector.reduce_sum(out=s[:], in_=ex[:], axis=X)
            inv = sp.tile([128, NT, 1], F32, tag="inv")
            nc.vector.reciprocal(out=inv[:], in_=s[:])
            pr = sp.tile([128, NT, E], F32, tag="pr")
            nc.vector.tensor_mul(out=pr[:], in0=ex[:], in1=inv[:].to_broadcast((128, NT, E)))
            nc.sync.dma_start(probs[:], pr[:])
    nc.compile()
    return nc


# ---------------------------------------------------------------- mlp
def _build_mlp_nc(tpl):
    """Fused 3-layer expert MLP; one slot per template entry, bf16 matmuls.

    Slot j: xt{j} [128, D/128, S] bf16, w1_{j} [128, D/128, H], w2_{j}
    [128, H/128, H], w3_{j} [128, H/128, O] bf16, wv{j} [128, S] f32
    (per-token combine weight, pre-broadcast), output yt{j} [128, O/128, S]
    bf16 (already scaled by wv).
    """
    nc = _nc()
    t = {}
    for j, S in enumerate(tpl):
        # weights pre-tiled host-side: [m][p, k, col] so each m-tile DMA is
        # one contiguous 2KB-per-partition transfer (no small-descriptor
        # penalty)
        t[f"xt{j}"] = nc.dram_tensor(f"xt{j}", [128, D // 128, S], BF16, kind="ExternalInput")
        t[f"w1_{j}"] = nc.dram_tensor(f"w1_{j}", [H // 128, 128, D // 128, 128], BF16, kind="ExternalInput")
        t[f"w2_{j}"] = nc.dram_tensor(f"w2_{j}", [H // 128, 128, H // 128, 128], BF16, kind="ExternalInput")
        t[f"w3_{j}"] = nc.dram_tensor(f"w3_{j}", [O // 128, 128, H // 128, 128], BF16, kind="ExternalInput")
        t[f"wv{j}"] = nc.dram_tensor(f"wv{j}", [128, S], F32, kind="ExternalInput")
        t[f"yt{j}"] = nc.dram_tensor(f"yt{j}", [128, O // 128, S], BF16, kind="ExternalOutput")
    Smax = max(tpl)
    with tile.TileContext(nc) as tc:
        with (
            tc.tile_pool(name="x", bufs=2) as xp,
            tc.tile_pool(name="w1", bufs=3) as w1p,
            tc.tile_pool(name="w2", bufs=3) as w2p,
            tc.tile_pool(name="w3", bufs=3) as w3p,
            tc.tile_pool(name="h", bufs=2) as hp,
            tc.tile_pool(name="wv", bufs=2) as wvp,
            tc.tile_pool(name="y", bufs=4) as yp,
            tc.tile_pool(name="ps", bufs=6, space="PSUM") as pp,
        ):
            for j, S in enumerate(tpl):
                chunks = [(o, min(512, S - o)) for o in range(0, S, 512)]
                xsb = xp.tile([128, D // 128, Smax], BF16, tag="x")
                for k in range(D // 128):
                    nc.sync.dma_start(xsb[:, k, :S], t[f"xt{j}"][:, k])
                wv = wvp.tile([128, Smax], F32, tag="wv")
                nc.sync.dma_start(wv[:, :S], t[f"wv{j}"][:])
                h1 = hp.tile([128, H // 128, Smax], BF16, tag="h1")
                for m in range(H // 128):
                    w = w1p.tile([128, D // 128, 128], BF16, tag="w1")
                    nc.sync.dma_start(w[:], t[f"w1_{j}"][m])
                    for o, sz in chunks:
                        ps = pp.tile([128, 512], F32, tag="ps", name="ps")[:, :sz]
                        for k in range(D // 128):
                            nc.tensor.matmul(
                                ps,
                                lhsT=w[:, k],
                                rhs=xsb[:, k, o : o + sz],
                                start=(k == 0),
                                stop=(k == D // 128 - 1),
                            )
                        nc.scalar.activation(h1[:, m, o : o + sz], ps, ACT.Relu)
                h2 = hp.tile([128, H // 128, Smax], BF16, tag="h2")
                for m in range(H // 128):
                    w = w2p.tile([128, H // 128, 128], BF16, tag="w2")
                    nc.sync.dma_start(w[:], t[f"w2_{j}"][m])
                    for o, sz in chunks:
                        ps = pp.tile([128, 512], F32, tag="ps", name="ps")[:, :sz]
                        for k in range(H // 128):
                            nc.tensor.matmul(
                                ps,
                                lhsT=w[:, k],
                                rhs=h1[:, k, o : o + sz],
                                start=(k == 0),
                                stop=(k == H // 128 - 1),
                            )
                        nc.scalar.activation(h2[:, m, o : o + sz], ps, ACT.Relu)
                for m in range(O // 128):
                    w = w3p.tile([128, H // 128, 128], BF16, tag="w3")
                    nc.sync.dma_start(w[:], t[f"w3_{j}"][m])
                    for o, sz in chunks:
                        ps = pp.tile([128, 512], F32, tag="ps", name="ps")[:, :sz]
                        for k in range(H // 128):
                            nc.tensor.matmul(
                                ps,
                                lhsT=w[:, k],
                                rhs=h2[:, k, o : o + sz],
                                start=(k == 0),
                                stop=(k == H // 128 - 1),
                            )
                        y = yp.tile([128, 512], BF16, tag="y", name="y")[:, :sz]
                        nc.vector.tensor_mul(out=y, in0=ps, in1=wv[:, o : o + sz])
                        nc.sync.dma_start(t[f"yt{j}"][:, m, o : o + sz], y)
    nc.compile()
    return nc


# ---------------------------------------------------------------- fp8 mlp
# 3-term residual-split fp8 matmuls in DoubleRow perf mode (2 contraction
# rows per PE pass): W ~= (Whi + Wlo)/sw, x ~= (xhi + xlo)/sx, and
# W^T x ~= (Whi^T xhi + Wlo^T xhi + Whi^T xlo) / (sw sx), dropping only the
# lo*lo term (~1e-3 relative). Measured end-to-end rel err ~3.4e-3.
FP8 = mybir.dt.float8e4
E4M3 = ml_dtypes.float8_e4m3
SX, SH = 32.0, 32.0
SW1, SW2, SW3 = 512.0, 1024.0, 1024.0
USE_FP8 = True


def _build_mlp_fp8_nc(tpl):
    nc = _nc()
    t = {}
    KD, KH = D // 128, H // 128
    for j, S in enumerate(tpl):
        t[f"xhi{j}"] = nc.dram_tensor(f"xhi{j}", [128, KD, S], FP8, kind="ExternalInput")
        t[f"xlo{j}"] = nc.dram_tensor(f"xlo{j}", [128, KD, S], FP8, kind="ExternalInput")
        for L, KT, MT in (("1", KD, KH), ("2", KH, KH), ("3", KH, O // 128)):
            # hi and lo halves concatenated along k so one DMA loads both
            # (DMA issue costs ~650ns of SEQ+HWDGE each; count matters)
            t[f"w{L}_{j}"] = nc.dram_tensor(
                f"w{L}_{j}", [MT, 128, 2 * KT, 128], FP8, kind="ExternalInput"
            )
        t[f"wv{j}"] = nc.dram_tensor(f"wv{j}", [128, S], F32, kind="ExternalInput")
        t[f"yt{j}"] = nc.dram_tensor(f"yt{j}", [128, O // 128, S], BF16, kind="ExternalOutput")
    Smax = max(tpl)
    DR = mybir.MatmulPerfMode.DoubleRow

    with tile.TileContext(nc) as tc:
        with (
            tc.tile_pool(name="x", bufs=2) as xp,
            tc.tile_pool(name="w1", bufs=4) as w1p,
            tc.tile_pool(name="w2", bufs=6) as w2p,
            tc.tile_pool(name="w3", bufs=4) as w3p,
            tc.tile_pool(name="h", bufs=1) as hp,
            tc.tile_pool(name="hb", bufs=4) as hbp,
            tc.tile_pool(name="wv", bufs=2) as wvp,
            tc.tile_pool(name="y", bufs=3) as yp,
            tc.tile_pool(name="ps", bufs=6, space="PSUM") as pp,
            tc.tile_pool(name="warm", bufs=1, space="PSUM") as wmp,
        ):
            warm = wmp.tile([128, 128], F32, tag="warm")
            for j, S in enumerate(tpl):
                chunks = [(o, min(512, S - o)) for o in range(0, S, 512)]
                # split x loads so the first matmul group's operands (k-tiles
                # 0..1 of xhi then xlo) land first; wv is deferred to L3
                xh = xp.tile([128, KD, Smax], FP8, tag="xh")
                xl = xp.tile([128, KD, Smax], FP8, tag="xl")
                nc.sync.dma_start(xh[:, :2, :S], t[f"xhi{j}"][:, :2])
                nc.sync.dma_start(xh[:, 2:, :S], t[f"xhi{j}"][:, 2:])
                nc.sync.dma_start(xl[:, :2, :S], t[f"xlo{j}"][:, :2])
                nc.sync.dma_start(xl[:, 2:, :S], t[f"xlo{j}"][:, 2:])
                if j == 0:
                    # PE p-state warmup on the first-arriving x tile
                    for _ in range(28):
                        nc.tensor.matmul(
                            warm[:], lhsT=xh[:, 0, :128], rhs=xh[:, 0, :128],
                            start=True, stop=True,
                        )
                wv = wvp.tile([128, Smax], F32, tag="wv")

                def layer(L, KT, MT, ihi, ilo, wpool, out_cb):
                    for m in range(MT):
                        w = wpool.tile([128, 2 * KT, 128], FP8, tag=f"w{L}")
                        nc.sync.dma_start(w[:], t[f"w{L}_{j}"][m])
                        whi, wlo = w[:, :KT], w[:, KT:]
                        terms = [(whi, ihi), (wlo, ihi), (whi, ilo)]
                        for o, sz in chunks:
                            ps = pp.tile([128, 512], F32, tag="ps", name="ps")[:, :sz]
                            for ti, (wt, it) in enumerate(terms):
                                for k2 in range(KT // 2):
                                    nc.tensor.matmul(
                                        ps,
                                        lhsT=wt[:, 2 * k2 : 2 * k2 + 2, :],
                                        rhs=it[:, 2 * k2 : 2 * k2 + 2, o : o + sz],
                                        start=(ti == 0 and k2 == 0),
                                        stop=(ti == 2 and k2 == KT // 2 - 1),
                                        perf_mode=DR,
                                    )
                            out_cb(m, o, sz, ps)

                h1hi = hp.tile([128, KH, Smax], FP8, tag="h1hi")
                h1lo = hp.tile([128, KH, Smax], FP8, tag="h1lo")

                def evict1(m, o, sz, ps):
                    hb = hbp.tile([128, 512], BF16, tag="hb", name="hb")[:, :sz]
                    nc.scalar.activation(hb, ps, ACT.Relu, scale=SH / (SW1 * SX))
                    nc.scalar.activation(h1hi[:, m, o : o + sz], hb, ACT.Copy)
                    nc.vector.tensor_tensor(
                        h1lo[:, m, o : o + sz], hb, h1hi[:, m, o : o + sz],
                        mybir.AluOpType.subtract,
                    )

                layer("1", KD, KH, xh, xl, w1p, evict1)

                h2hi = hp.tile([128, KH, Smax], FP8, tag="h2hi")
                h2lo = hp.tile([128, KH, Smax], FP8, tag="h2lo")

                def evict2(m, o, sz, ps):
                    hb = hbp.tile([128, 512], BF16, tag="hb", name="hb")[:, :sz]
                    nc.scalar.activation(hb, ps, ACT.Relu, scale=1.0 / SW2)
                    nc.scalar.activation(h2hi[:, m, o : o + sz], hb, ACT.Copy)
                    nc.vector.tensor_tensor(
                        h2lo[:, m, o : o + sz], hb, h2hi[:, m, o : o + sz],
                        mybir.AluOpType.subtract,
                    )

                layer("2", KH, KH, h1hi, h1lo, w2p, evict2)

                nc.sync.dma_start(wv[:, :S], t[f"wv{j}"][:])
                ycur = [None]

                def evict3(m, o, sz, ps):
                    if ycur[0] is None:
                        ycur[0] = yp.tile([128, Smax], BF16, tag="y", name="y")
                    nc.vector.tensor_mul(
                        out=ycur[0][:, o : o + sz], in0=ps, in1=wv[:, o : o + sz]
                    )
                    if o + sz >= S:  # last chunk of this m: flush one DMA
                        nc.sync.dma_start(t[f"yt{j}"][:, m, :S], ycur[0][:, :S])
                        ycur[0] = None

                layer("3", KH, O // 128, h2hi, h2lo, w3p, evict3)
    nc.compile()
    return nc


# ---------------------------------------------------------------- combine
def _build_comb_nc(R):
    nc = _nc()
    NT = TPC // 128
    yall = nc.dram_tensor("yall", [R, O], BF16, kind="ExternalInput")
    i0 = nc.dram_tensor("i0", [128, NT], I32, kind="ExternalInput")
    i1 = nc.dram_tensor("i1", [128, NT], I32, kind="ExternalInput")
    out = nc.dram_tensor("out", [128, NT, O], BF16, kind="ExternalOutput")
    with tile.TileContext(nc) as tc:
        with (
            tc.tile_pool(name="big", bufs=4) as gp,
            tc.tile_pool(name="idx", bufs=1) as ip,
        ):
            i0_t = ip.tile([128, NT], I32)
            i1_t = ip.tile([128, NT], I32)
            nc.sync.dma_start(i0_t[:], i0[:])
            nc.sync.dma_start(i1_t[:], i1[:])
            # per-tile single-index gathers (the runtime only supports one
            # index per partition per indirect DMA). Independent g0/g1 tiles
            # keep the Pool SWDGE queue saturated; DVE adds and per-tile
            # output writes pipeline underneath.
            for i in range(NT):
                g0 = gp.tile([128, O], BF16, tag="g0", name="g0")
                g1 = gp.tile([128, O], BF16, tag="g1", name="g1")
                nc.gpsimd.indirect_dma_start(
                    out=g0[:],
                    out_offset=None,
                    in_=yall[:],
                    in_offset=bass.IndirectOffsetOnAxis(ap=i0_t[:, i : i + 1], axis=0),
                )
                nc.gpsimd.indirect_dma_start(
                    out=g1[:],
                    out_offset=None,
                    in_=yall[:],
                    in_offset=bass.IndirectOffsetOnAxis(ap=i1_t[:, i : i + 1], axis=0),
                )
                nc.vector.tensor_add(out=g0[:], in0=g0[:], in1=g1[:])
                nc.sync.dma_start(out[:, i], g0[:])
    nc.compile()
    return nc


# ---------------------------------------------------------------- packing
def _try_assign(units, tpl_u, slack_cap):
    """Cover each expert's unit count with slots from 8x tpl_u inventory.

    units/tpl_u are in 128-token units. slack_cap bounds per-expert
    over-allocation (in units). Returns {expert: {size: n}} or None.
    """
    inv = {}
    for s in tpl_u:
        inv[s] = inv.get(s, 0) + NCORES
    sizes = sorted(inv, reverse=True)
    experts = sorted(range(len(units)), key=lambda e: -units[e])
    budget = [0]

    def covers(c, i, slack, out, cur):
        if len(out) >= 40 or budget[0] > 100000:
            return
        budget[0] += 1
        if c <= 0:
            if -c <= slack:
                out.append(dict(cur))
            return
        if i >= len(sizes):
            return
        s = sizes[i]
        hi = min(inv[s], (c + slack) // s)
        for n in range(hi, -1, -1):
            if i == len(sizes) - 1 and n * s < c:
                break  # last size can't cover the remainder
            if n:
                cur[s] = n
            covers(c - n * s, i + 1, slack, out, cur)
            cur.pop(s, None)

    def dfs(idx):
        budget[0] += 1
        if budget[0] > 100000:
            return None
        if idx == len(experts):
            return {}
        e = experts[idx]
        c = int(units[e])
        if c == 0:
            rest = dfs(idx + 1)
            if rest is not None:
                rest[e] = {}
            return rest
        out = []
        covers(c, 0, slack_cap, out, {})
        out.sort(key=lambda d: (sum(s * n for s, n in d.items()) - c, sum(d.values())))
        for cov in out[:16]:
            if not all(inv[s] >= n for s, n in cov.items()):
                continue
            for s, n in cov.items():
                inv[s] -= n
            rest = dfs(idx + 1)
            if rest is not None:
                rest[e] = cov
                return rest
            for s, n in cov.items():
                inv[s] += n
        return None

    return dfs(0)


def _solve_packing(counts):
    """Pick per-core slot-size template (uniform across cores) + expert cover.

    Works in 128-token units; returns (template_in_tokens, assign) where
    assign maps expert -> {slot_size_tokens: n_slots}.
    """
    G = 32  # slot-size granularity in tokens
    units = [-(-int(c) // G) for c in counts]
    U0 = max(1, -(-sum(units) // NCORES))
    maxu = max(units + [1])

    def partitions(total, maxpart, maxparts):
        if total == 0:
            yield ()
            return
        if maxparts == 0:
            return
        for p in range(min(total, maxpart), 0, -1):
            for rest in partitions(total - p, p, maxparts - 1):
                yield (p,) + rest

    for U in range(U0, U0 + 24):
        tpls = sorted(
            {t for t in partitions(U, 1024 // G, 5)},
            key=lambda t: (len(t), -min(t), tuple(-v for v in t)),
        )[:64]
        for slack in (0, 1, 2, 4):
            for tpl_u in tpls:
                asg = _try_assign(units, tpl_u, slack)
                if asg is not None:
                    tpl = tuple(s * G for s in tpl_u)
                    return tpl, {
                        e: {s * G: n for s, n in cov.items()}
                        for e, cov in asg.items()
                    }
    # retry coarser granularity before the uniform fallback
    for G2 in (64, 128):
        U0b = max(1, -(-sum(-(-int(c) // G2) for c in counts) // NCORES))
        for U in range(U0b, U0b + 24):
            tpls = sorted(
                {t for t in partitions(U, 1024 // G2, 5)},
                key=lambda t: (len(t), -min(t), tuple(-v for v in t)),
            )[:64]
            for slack in (0, 1, 2, 4):
                for tpl_u in tpls:
                    asg = _try_assign([-(-int(c) // G2) for c in counts], tpl_u, slack)
                    if asg is not None:
                        return tuple(s * G2 for s in tpl_u), {
                            e: {s * G2: n for s, n in cov.items()}
                            for e, cov in asg.items()
                        }
    # ultimate fallback: uniform 2 slots covering the largest expert
    S = max(128, (int(max(counts)) // 2 // 128 + 1) * 128)
    while True:
        if sum(-(-int(c) // S) for c in counts if c) <= 2 * NCORES:
            asg = {e: {S: -(-int(c) // S)} for e, c in enumerate(counts) if c}
            return (S, S), asg
        S += 128
    raise RuntimeError("no packing found")


# ---------------------------------------------------------------- kernel
def kernel(x, W1, b1, W2, b2, W3, b3, Wg1, bg1, Wg2, bg2, top_k):
    x = np.asarray(x, np.float32)
    W1 = np.asarray(W1, np.float32)
    W2 = np.asarray(W2, np.float32)
    W3 = np.asarray(W3, np.float32)
    Wg1 = np.asarray(Wg1, np.float32)
    Wg2 = np.asarray(Wg2, np.float32)
    assert int(np.asarray(top_k)) == 2
    for b in (b1, b2, b3, bg1, bg2):
        assert not np.any(np.asarray(b)), "nonzero biases unsupported"

    core_ids = list(range(NCORES))
    NT = TPC // 128

    # ---------------- gate: logits + softmax on device ----------------
    if "gate" not in _CACHE:
        _CACHE["gate"] = _build_gate_nc()
    xT32 = np.ascontiguousarray(x.T)  # [D, N] fp32
    xT = xT32.astype(BF)  # bf16 hi part, also the expert-MLP input
    xTlo = (xT32 - xT.astype(np.float32)).astype(BF)  # bf16 residual
    wg1p = np.zeros((D, 128), np.float32)
    wg1p[:, :64] = Wg1
    wg2p = np.zeros((128, 128), np.float32)
    wg2p[:64, :E] = Wg2
    wg1hi = _pmn(wg1p)
    wg1lo = _pmn(wg1p - wg1hi.transpose(1, 0, 2).reshape(D, 128).astype(np.float32))

    def _ptile(a, c):  # [D, N] -> per-core [128, D/128, TPC]
        return np.ascontiguousarray(
            a[:, c * TPC : (c + 1) * TPC].reshape(D // 128, 128, TPC).transpose(1, 0, 2)
        )

    in1 = [
        {
            "xhi": _ptile(xT, c),
            "xlo": _ptile(xTlo, c),
            "wg1hi": wg1hi,
            "wg1lo": wg1lo,
            "wg2": wg2p,
        }
        for c in core_ids
    ]
    res1 = run_bass_kernel_spmd(_CACHE["gate"], in1, core_ids).results
    # probs[p, i, e] -> token c*TPC + i*128 + p
    probs = np.concatenate(
        [res1[c]["probs"].transpose(1, 0, 2).reshape(TPC, E) for c in core_ids], axis=0
    ).astype(np.float32)  # [N, E]

    # ---------------- host routing (comparisons/indexing only) ----------------
    top2 = np.argsort(-probs, axis=1, kind="stable")[:, :2]  # [N, 2]
    e0s, e1s = top2[:, 0], top2[:, 1]
    expert_lists = [np.nonzero((top2 == e).any(axis=1))[0] for e in range(E)]
    counts = np.array([len(t) for t in expert_lists])

    tpl, asg = _solve_packing(counts)
    C = sum(tpl)

    # slot positions per size: (core, slot_idx) pools
    pos_pool = {}
    for jj, s in enumerate(tpl):
        pos_pool.setdefault(s, []).extend((c, jj) for c in core_ids)
    slot_off = {}  # (core, j) -> global row offset in yall
    for c in core_ids:
        off = c * C
        for jj, s in enumerate(tpl):
            slot_off[(c, jj)] = off
            off += s
    # assign slots to experts, chop token lists
    slot_map = {}  # (core, j) -> (expert, token_array)
    glob_row = np.zeros((N, E), np.int64)
    for e in range(E):
        tl = expert_lists[e]
        claims = []
        for s in sorted(asg.get(e, {}), reverse=True):
            for _ in range(asg[e][s]):
                claims.append((s, pos_pool[s].pop()))
        cum = 0
        for s, posn in claims:
            chunk = tl[cum : cum + s]
            slot_map[posn] = (e, chunk)
            glob_row[chunk, e] = slot_off[posn] + np.arange(len(chunk))
            cum += s
        assert cum >= len(tl), f"expert {e} not fully covered"
    # leftover slots empty
    for s, pool in pos_pool.items():
        for posn in pool:
            slot_map[posn] = (0, np.zeros(0, np.int64))

    w_tok = np.take_along_axis(probs, top2, axis=1)  # [N, 2] combine weights

    # ---------------- mlp: fused expert MLP on device ----------------
    key2 = (("emlp8" if USE_FP8 else "emlp"), tpl)
    if key2 not in _CACHE:
        _CACHE[key2] = _build_mlp_fp8_nc(tpl) if USE_FP8 else _build_mlp_nc(tpl)
    Wb = {}

    def _mtile(a):
        # [128, K/128, M] -> [M/128, 128, K/128, 128] (m-tile contiguous)
        P, KT, M = a.shape
        return np.ascontiguousarray(
            a.reshape(P, KT, M // 128, 128).transpose(2, 0, 1, 3)
        )

    def _split8(W, sw):
        Ws = W * sw
        hi = Ws.astype(E4M3)
        lo = (Ws - hi.astype(np.float32)).astype(E4M3)
        # concat hi and lo along the k-tile axis: [MT, 128, 2*KT, 128]
        return np.ascontiguousarray(
            np.concatenate([_mtile(_pmn(hi, E4M3)), _mtile(_pmn(lo, E4M3))], axis=2)
        )

    def wts(e):
        if e not in Wb:
            if USE_FP8:
                Wb[e] = (
                    _split8(W1[e], SW1),
                    _split8(W2[e], SW2),
                    _split8(W3[e], SW3),
                )
            else:
                Wb[e] = (_mtile(_pmn(W1[e])), _mtile(_pmn(W2[e])), _mtile(_pmn(W3[e])))
        return Wb[e]

    if USE_FP8:
        Xs = xT32 * SX
        xhi8 = Xs.astype(E4M3)
        xlo8 = (Xs - xhi8.astype(np.float32)).astype(E4M3)

    wmask = np.zeros((N, E), np.float32)
    wmask[np.arange(N), e0s] = w_tok[:, 0]
    wmask[np.arange(N), e1s] = w_tok[:, 1]

    def _pt(a, S):  # [D, S] -> [128, D/128, S]
        return np.ascontiguousarray(a.reshape(D // 128, 128, S).transpose(1, 0, 2))

    in2 = []
    for c in core_ids:
        d = {}
        for jj, S in enumerate(tpl):
            e, chunk = slot_map[(c, jj)]
            padded = np.zeros(S, np.int64)
            padded[: len(chunk)] = chunk
            wv = np.zeros(S, np.float32)
            wv[: len(chunk)] = wmask[chunk, e]
            if USE_FP8:
                d[f"xhi{jj}"] = _pt(xhi8[:, padded], S)
                d[f"xlo{jj}"] = _pt(xlo8[:, padded], S)
                d[f"w1_{jj}"], d[f"w2_{jj}"], d[f"w3_{jj}"] = wts(e)
                wv = wv / (SW3 * SH)
            else:
                w1p, w2p, w3p = wts(e)
                d[f"xt{jj}"] = _pt(xT[:, padded], S)
                d[f"w1_{jj}"] = w1p
                d[f"w2_{jj}"] = w2p
                d[f"w3_{jj}"] = w3p
            d[f"wv{jj}"] = np.ascontiguousarray(
                np.broadcast_to(wv[None, :], (128, S))
            )
        in2.append(d)
    res2 = run_bass_kernel_spmd(_CACHE[key2], in2, core_ids).results

    R = NCORES * C
    yall = np.empty((R, O), BF)
    for c in core_ids:
        for jj, S in enumerate(tpl):
            yt = res2[c][f"yt{jj}"]  # [128, O/128, S]
            off = slot_off[(c, jj)]
            yall[off : off + S] = yt.transpose(2, 1, 0).reshape(S, O)

    # ---------------- combine: gather 2 scaled rows + add ----------------
    key3 = ("comb", R)
    if key3 not in _CACHE:
        _CACHE[key3] = _build_comb_nc(R)
    g0 = glob_row[np.arange(N), e0s].astype(np.int32)
    g1 = glob_row[np.arange(N), e1s].astype(np.int32)
    in3 = []
    for c in core_ids:
        sl = slice(c * TPC, (c + 1) * TPC)
        in3.append(
            {
                "yall": yall,
                "i0": np.ascontiguousarray(g0[sl].reshape(NT, 128).T),
                "i1": np.ascontiguousarray(g1[sl].reshape(NT, 128).T),
            }
        )
    res3 = run_bass_kernel_spmd(_CACHE[key3], in3, core_ids).results
    out = np.concatenate(
        [
            res3[c]["out"].transpose(1, 0, 2).reshape(TPC, O).astype(np.float32)
            for c in core_ids
        ],
        axis=0,
    )
    return out


# revision 32
# speedup vs baseline: 1.0120x; 1.0002x over previous
"""MoE (8 experts, top-2) Trainium2 Bass kernel, expert-parallel over 8 cores.

Pipeline (all FLOPs on device):
  gate   : gating logits + softmax for all tokens (data-parallel over cores),
           output token-major probs [N, E]
  host   : top-2 selection + slot packing (comparisons/indexing only)
  mlp    : per-core fused 3-layer expert MLP on gathered tokens (bf16 matmuls,
           h1/h2 resident in SBUF, per-token gate weight folded into L3 evict)
  combine: per-token gather of its two scaled expert rows + add
"""

import numpy as np
import ml_dtypes

import jax

jax.config.update("jax_compilation_cache_dir", "/tmp/jax_comp_cache")
jax.config.update("jax_persistent_cache_min_entry_size_bytes", -1)
jax.config.update("jax_persistent_cache_min_compile_time_secs", 0)

import concourse.bass as bass
import concourse.mybir as mybir
import concourse.tile as tile
from concourse import bacc
from concourse.bass_utils import run_bass_kernel_spmd

N, D, H, O, E = 8192, 1024, 2048, 1024, 8
NCORES = 8
TPC = N // NCORES  # tokens per core for gate/combine phases
F32 = mybir.dt.float32
BF16 = mybir.dt.bfloat16
I32 = mybir.dt.int32
BF = ml_dtypes.bfloat16
ACT = mybir.ActivationFunctionType

_CACHE = {}


def _nc():
    return bacc.Bacc(None, target_bir_lowering=False, debug=True)


def _pmn(a, dtype=BF):
    """[K, M] row-major -> [128, K/128, M] with row k = m*128 + p."""
    K, M = a.shape
    return np.ascontiguousarray(
        a.reshape(K // 128, 128, M).transpose(1, 0, 2).astype(dtype)
    )


# ---------------------------------------------------------------- gate
def _build_gate_nc():
    # Logits need fp32-class accuracy so top-2 selection and combine weights
    # match the fp32 reference on near-tied gates (bf16 logits flip ~0.6% of
    # tokens' second expert -> 5e-2 output error). L1 uses a 3-term bf16
    # residual split (err ~2^-17), L2 runs in true fp32 (4 cyc/row, tiny).
    nc = _nc()
    NT = TPC // 128  # token tiles per core
    xhi = nc.dram_tensor("xhi", [128, D // 128, TPC], BF16, kind="ExternalInput")
    xlo = nc.dram_tensor("xlo", [128, D // 128, TPC], BF16, kind="ExternalInput")
    wg1hi = nc.dram_tensor("wg1hi", [128, D // 128, 128], BF16, kind="ExternalInput")
    wg1lo = nc.dram_tensor("wg1lo", [128, D // 128, 128], BF16, kind="ExternalInput")
    wg2 = nc.dram_tensor("wg2", [128, 128], F32, kind="ExternalInput")
    probs = nc.dram_tensor("probs", [128, NT, E], F32, kind="ExternalOutput")
    X = mybir.AxisListType.X
    with tile.TileContext(nc) as tc:
        with (
            tc.tile_pool(name="w", bufs=1) as wp,
            tc.tile_pool(name="x", bufs=2) as xp,
            tc.tile_pool(name="g", bufs=1) as gp,
            tc.tile_pool(name="s", bufs=2) as sp,
            tc.tile_pool(name="ps", bufs=3, space="PSUM") as pp,
            tc.tile_pool(name="warm", bufs=1, space="PSUM") as wmp,
        ):
            whi = wp.tile([128, D // 128, 128], BF16)
            wlo = wp.tile([128, D // 128, 128], BF16)
            wg2t = wp.tile([128, 128], F32)
            nc.sync.dma_start(whi[:], wg1hi[:])
            nc.sync.dma_start(wlo[:], wg1lo[:])
            nc.sync.dma_start(wg2t[:], wg2[:])
            # PE p-state warmup: the cost model ramps 0.65->1.2->2.4 GHz over
            # 3us of continuous PE activity; dummy matmuls on the (early)
            # weight tile keep the clock ramping while x streams in.
            warm = wmp.tile([128, 128], F32, tag="warm")
            for _ in range(44):
                nc.tensor.matmul(warm[:], lhsT=whi[:, 0], rhs=whi[:, 1], start=True, stop=True)
            # L1: g1 = relu(Wg1^T x) ~= relu((Whi+Wlo)^T xhi + Whi^T xlo)
            # x loaded per 512-token chunk so DMA pipelines with compute
            g1 = gp.tile([128, NT // 4, 512], F32)  # [feat, chunk, token]
            for ci, i in enumerate(range(0, TPC, 512)):
                xh = xp.tile([128, D // 128, 512], BF16, tag="xh", name="xh")
                xl = xp.tile([128, D // 128, 512], BF16, tag="xl", name="xl")
                nc.sync.dma_start(xh[:], xhi[:, :, i : i + 512])
                nc.sync.dma_start(xl[:], xlo[:, :, i : i + 512])
                ps = pp.tile([128, 512], F32, tag="ps1")
                terms = [(whi, xh), (wlo, xh), (whi, xl)]
                for ti, (wt, xt_) in enumerate(terms):
                    for k in range(D // 128):
                        nc.tensor.matmul(
                            ps[:],
                            lhsT=wt[:, k],
                            rhs=xt_[:, k],
                            start=(ti == 0 and k == 0),
                            stop=(ti == 2 and k == D // 128 - 1),
                        )
                nc.scalar.activation(g1[:, ci], ps[:], ACT.Relu)
            # L2 (token-major, fp32): logitsT[t, e] = sum_f g1[f, t] Wg2[f, e]
            ex = sp.tile([128, NT, E], F32, tag="ex")
            for i in range(NT):
                ps = pp.tile([128, 128], F32, tag="ps2")
                nc.tensor.matmul(
                    ps[:],
                    lhsT=g1[:, i // 4, (i % 4) * 128 : (i % 4 + 1) * 128],
                    rhs=wg2t[:],
                    start=True,
                    stop=True,
                )
                nc.scalar.activation(ex[:, i], ps[:, :E], ACT.Exp)
            # softmax + output in two halves: the first half's chain runs
            # while L2 tiles 4-7 are still on the PE, shortening the tail
            for hh in range(2):
                exh = ex[:, hh * 4 : hh * 4 + 4]
                s = sp.tile([128, 4, 1], F32, tag="s", name="s")
                nc.vector.reduce_sum(out=s[:], in_=exh, axis=X)
                inv = sp.tile([128, 4, 1], F32, tag="inv", name="inv")
                nc.vector.reciprocal(out=inv[:], in_=s[:])
                pr = sp.tile([128, 4, E], F32, tag="pr", name="pr")
                nc.vector.tensor_mul(out=pr[:], in0=exh, in1=inv[:].to_broadcast((128, 4, E)))
                nc.sync.dma_start(probs[:, hh * 4 : hh * 4 + 4], pr[:])
    nc.compile()
    return nc


# ---------------------------------------------------------------- mlp
def _build_mlp_nc(tpl):
    """Fused 3-layer expert MLP; one slot per template entry, bf16 matmuls.

    Slot j: xt{j} [128, D/128, S] bf16, w1_{j} [128, D/128, H], w2_{j}
    [128, H/128, H], w3_{j} [128, H/128, O] bf16, wv{j} [128, S] f32
    (per-token combine weight, pre-broadcast), output yt{j} [128, O/128, S]
    bf16 (already scaled by wv).
    """
    nc = _nc()
    t = {}
    for j, S in enumerate(tpl):
        # weights pre-tiled host-side: [m][p, k, col] so each m-tile DMA is
        # one contiguous 2KB-per-partition transfer (no small-descriptor
        # penalty)
        t[f"xt{j}"] = nc.dram_tensor(f"xt{j}", [128, D // 128, S], BF16, kind="ExternalInput")
        t[f"w1_{j}"] = nc.dram_tensor(f"w1_{j}", [H // 128, 128, D // 128, 128], BF16, kind="ExternalInput")
        t[f"w2_{j}"] = nc.dram_tensor(f"w2_{j}", [H // 128, 128, H // 128, 128], BF16, kind="ExternalInput")
        t[f"w3_{j}"] = nc.dram_tensor(f"w3_{j}", [O // 128, 128, H // 128, 128], BF16, kind="ExternalInput")
        t[f"wv{j}"] = nc.dram_tensor(f"wv{j}", [128, S], F32, kind="ExternalInput")
        t[f"yt{j}"] = nc.dram_tensor(f"yt{j}", [128, O // 128, S], BF16, kind="ExternalOutput")
    Smax = max(tpl)
    with tile.TileContext(nc) as tc:
        with (
            tc.tile_pool(name="x", bufs=2) as xp,
            tc.tile_pool(name="w1", bufs=3) as w1p,
            tc.tile_pool(name="w2", bufs=3) as w2p,
            tc.tile_pool(name="w3", bufs=3) as w3p,
            tc.tile_pool(name="h", bufs=2) as hp,
            tc.tile_pool(name="wv", bufs=2) as wvp,
            tc.tile_pool(name="y", bufs=4) as yp,
            tc.tile_pool(name="ps", bufs=6, space="PSUM") as pp,
        ):
            for j, S in enumerate(tpl):
                chunks = [(o, min(512, S - o)) for o in range(0, S, 512)]
                xsb = xp.tile([128, D // 128, Smax], BF16, tag="x")
                for k in range(D // 128):
                    nc.sync.dma_start(xsb[:, k, :S], t[f"xt{j}"][:, k])
                wv = wvp.tile([128, Smax], F32, tag="wv")
                nc.sync.dma_start(wv[:, :S], t[f"wv{j}"][:])
                h1 = hp.tile([128, H // 128, Smax], BF16, tag="h1")
                for m in range(H // 128):
                    w = w1p.tile([128, D // 128, 128], BF16, tag="w1")
                    nc.sync.dma_start(w[:], t[f"w1_{j}"][m])
                    for o, sz in chunks:
                        ps = pp.tile([128, 512], F32, tag="ps", name="ps")[:, :sz]
                        for k in range(D // 128):
                            nc.tensor.matmul(
                                ps,
                                lhsT=w[:, k],
                                rhs=xsb[:, k, o : o + sz],
                                start=(k == 0),
                                stop=(k == D // 128 - 1),
                            )
                        nc.scalar.activation(h1[:, m, o : o + sz], ps, ACT.Relu)
                h2 = hp.tile([128, H // 128, Smax], BF16, tag="h2")
                for m in range(H // 128):
                    w = w2p.tile([128, H // 128, 128], BF16, tag="w2")
                    nc.sync.dma_start(w[:], t[f"w2_{j}"][m])
                    for o, sz in chunks:
                        ps = pp.tile([128, 512], F32, tag="ps", name="ps")[:, :sz]
                        for k in range(H // 128):
                            nc.tensor.matmul(
                                ps,
                                lhsT=w[:, k],
                                rhs=h1[:, k, o : o + sz],
                                start=(k == 0),
                                stop=(k == H // 128 - 1),
                            )
                        nc.scalar.activation(h2[:, m, o : o + sz], ps, ACT.Relu)
                for m in range(O // 128):
                    w = w3p.tile([128, H // 128, 128], BF16, tag="w3")
                    nc.sync.dma_start(w[:], t[f"w3_{j}"][m])
                    for o, sz in chunks:
                        ps = pp.tile([128, 512], F32, tag="ps", name="ps")[:, :sz]
                        for k in range(H // 128):
                            nc.tensor.matmul(
                                ps,
                                lhsT=w[:, k],
                                rhs=h2[:, k, o : o + sz],
                                start=(k == 0),
                                stop=(k == H // 128 - 1),
                            )
                        y = yp.tile([128, 512], BF16, tag="y", name="y")[:, :sz]
                        nc.vector.tensor_mul(out=y, in0=ps, in1=wv[:, o : o + sz])
                        nc.sync.dma_start(t[f"yt{j}"][:, m, o : o + sz], y)
    nc.compile()
    return nc


# ---------------------------------------------------------------- fp8 mlp
# 3-term residual-split fp8 matmuls in DoubleRow perf mode (2 contraction
# rows per PE pass): W ~= (Whi + Wlo)/sw, x ~= (xhi + xlo)/sx, and
# W^T x ~= (Whi^T xhi + Wlo^T xhi + Whi^T xlo) / (sw sx), dropping only the
# lo*lo term (~1e-3 relative). Measured end-to-end rel err ~3.4e-3.
FP8 = mybir.dt.float8e4
E4M3 = ml_dtypes.float8_e4m3
SX, SH = 32.0, 32.0
SW1, SW2, SW3 = 512.0, 1024.0, 1024.0
USE_FP8 = True


def _build_mlp_fp8_nc(tpl):
    nc = _nc()
    t = {}
    KD, KH = D // 128, H // 128
    for j, S in enumerate(tpl):
        t[f"xhi{j}"] = nc.dram_tensor(f"xhi{j}", [128, KD, S], FP8, kind="ExternalInput")
        t[f"xlo{j}"] = nc.dram_tensor(f"xlo{j}", [128, KD, S], FP8, kind="ExternalInput")
        for L, KT, MT in (("1", KD, KH), ("2", KH, KH), ("3", KH, O // 128)):
            # hi and lo halves concatenated along k so one DMA loads both
            # (DMA issue costs ~650ns of SEQ+HWDGE each; count matters)
            t[f"w{L}_{j}"] = nc.dram_tensor(
                f"w{L}_{j}", [MT, 128, 2 * KT, 128], FP8, kind="ExternalInput"
            )
        t[f"wv{j}"] = nc.dram_tensor(f"wv{j}", [128, S], F32, kind="ExternalInput")
        t[f"yt{j}"] = nc.dram_tensor(f"yt{j}", [128, O // 128, S], BF16, kind="ExternalOutput")
    Smax = max(tpl)
    DR = mybir.MatmulPerfMode.DoubleRow

    with tile.TileContext(nc) as tc:
        with (
            tc.tile_pool(name="x", bufs=2) as xp,
            tc.tile_pool(name="w1", bufs=4) as w1p,
            tc.tile_pool(name="w2", bufs=6) as w2p,
            tc.tile_pool(name="w3", bufs=4) as w3p,
            tc.tile_pool(name="h", bufs=1) as hp,
            tc.tile_pool(name="hb", bufs=4) as hbp,
            tc.tile_pool(name="wv", bufs=2) as wvp,
            tc.tile_pool(name="y", bufs=3) as yp,
            tc.tile_pool(name="ps", bufs=6, space="PSUM") as pp,
            tc.tile_pool(name="warm", bufs=1, space="PSUM") as wmp,
        ):
            warm = wmp.tile([128, 128], F32, tag="warm")
            for j, S in enumerate(tpl):
                chunks = [(o, min(512, S - o)) for o in range(0, S, 512)]
                # split x loads so the first matmul group's operands (k-tiles
                # 0..1 of xhi then xlo) land first; wv is deferred to L3
                xh = xp.tile([128, KD, Smax], FP8, tag="xh")
                xl = xp.tile([128, KD, Smax], FP8, tag="xl")
                nc.sync.dma_start(xh[:, :2, :S], t[f"xhi{j}"][:, :2])
                nc.sync.dma_start(xh[:, 2:, :S], t[f"xhi{j}"][:, 2:])
                nc.sync.dma_start(xl[:, :2, :S], t[f"xlo{j}"][:, :2])
                nc.sync.dma_start(xl[:, 2:, :S], t[f"xlo{j}"][:, 2:])
                if j == 0:
                    # PE p-state warmup on the first-arriving x tile
                    for _ in range(28):
                        nc.tensor.matmul(
                            warm[:], lhsT=xh[:, 0, :128], rhs=xh[:, 0, :128],
                            start=True, stop=True,
                        )
                wv = wvp.tile([128, Smax], F32, tag="wv")

                def layer(L, KT, MT, ihi, ilo, wpool, out_cb):
                    for m in range(MT):
                        w = wpool.tile([128, 2 * KT, 128], FP8, tag=f"w{L}")
                        nc.sync.dma_start(w[:], t[f"w{L}_{j}"][m])
                        whi, wlo = w[:, :KT], w[:, KT:]
                        terms = [(whi, ihi), (wlo, ihi), (whi, ilo)]
                        for o, sz in chunks:
                            ps = pp.tile([128, 512], F32, tag="ps", name="ps")[:, :sz]
                            for ti, (wt, it) in enumerate(terms):
                                for k2 in range(KT // 2):
                                    nc.tensor.matmul(
                                        ps,
                                        lhsT=wt[:, 2 * k2 : 2 * k2 + 2, :],
                                        rhs=it[:, 2 * k2 : 2 * k2 + 2, o : o + sz],
                                        start=(ti == 0 and k2 == 0),
                                        stop=(ti == 2 and k2 == KT // 2 - 1),
                                        perf_mode=DR,
                                    )
                            out_cb(m, o, sz, ps)

                h1hi = hp.tile([128, KH, Smax], FP8, tag="h1hi")
                h1lo = hp.tile([128, KH, Smax], FP8, tag="h1lo")

                def evict1(m, o, sz, ps):
                    hb = hbp.tile([128, 512], BF16, tag="hb", name="hb")[:, :sz]
                    nc.scalar.activation(hb, ps, ACT.Relu, scale=SH / (SW1 * SX))
                    nc.scalar.activation(h1hi[:, m, o : o + sz], hb, ACT.Copy)
                    nc.vector.tensor_tensor(
                        h1lo[:, m, o : o + sz], hb, h1hi[:, m, o : o + sz],
                        mybir.AluOpType.subtract,
                    )

                layer("1", KD, KH, xh, xl, w1p, evict1)

                h2hi = hp.tile([128, KH, Smax], FP8, tag="h2hi")
                h2lo = hp.tile([128, KH, Smax], FP8, tag="h2lo")

                def evict2(m, o, sz, ps):
                    hb = hbp.tile([128, 512], BF16, tag="hb", name="hb")[:, :sz]
                    nc.scalar.activation(hb, ps, ACT.Relu, scale=1.0 / SW2)
                    nc.scalar.activation(h2hi[:, m, o : o + sz], hb, ACT.Copy)
                    nc.vector.tensor_tensor(
                        h2lo[:, m, o : o + sz], hb, h2hi[:, m, o : o + sz],
                        mybir.AluOpType.subtract,
                    )

                layer("2", KH, KH, h1hi, h1lo, w2p, evict2)

                nc.sync.dma_start(wv[:, :S], t[f"wv{j}"][:])
                ycur = [None]

                def evict3(m, o, sz, ps):
                    if ycur[0] is None:
                        ycur[0] = yp.tile([128, Smax], BF16, tag="y", name="y")
                    nc.vector.tensor_mul(
                        out=ycur[0][:, o : o + sz], in0=ps, in1=wv[:, o : o + sz]
                    )
                    if o + sz >= S:  # last chunk of this m: flush one DMA
                        nc.sync.dma_start(t[f"yt{j}"][:, m, :S], ycur[0][:, :S])
                        ycur[0] = None

                layer("3", KH, O // 128, h2hi, h2lo, w3p, evict3)
    nc.compile()
    return nc


# ---------------------------------------------------------------- combine
def _build_comb_nc(R):
    nc = _nc()
    NT = TPC // 128
    yall = nc.dram_tensor("yall", [R, O], BF16, kind="ExternalInput")
    i0 = nc.dram_tensor("i0", [128, NT], I32, kind="ExternalInput")
    i1 = nc.dram_tensor("i1", [128, NT], I32, kind="ExternalInput")
    out = nc.dram_tensor("out", [128, NT, O], BF16, kind="ExternalOutput")
    with tile.TileContext(nc) as tc:
        with (
            tc.tile_pool(name="big", bufs=4) as gp,
            tc.tile_pool(name="idx", bufs=1) as ip,
        ):
            i0_t = ip.tile([128, NT], I32)
            i1_t = ip.tile([128, NT], I32)
            nc.sync.dma_start(i0_t[:], i0[:])
            nc.sync.dma_start(i1_t[:], i1[:])
            # per-tile single-index gathers (the runtime only supports one
            # index per partition per indirect DMA). Independent g0/g1 tiles
            # keep the Pool SWDGE queue saturated; DVE adds and per-tile
            # output writes pipeline underneath.
            for i in range(NT):
                g0 = gp.tile([128, O], BF16, tag="g0", name="g0")
                g1 = gp.tile([128, O], BF16, tag="g1", name="g1")
                nc.gpsimd.indirect_dma_start(
                    out=g0[:],
                    out_offset=None,
                    in_=yall[:],
                    in_offset=bass.IndirectOffsetOnAxis(ap=i0_t[:, i : i + 1], axis=0),
                )
                nc.gpsimd.indirect_dma_start(
                    out=g1[:],
                    out_offset=None,
                    in_=yall[:],
                    in_offset=bass.IndirectOffsetOnAxis(ap=i1_t[:, i : i + 1], axis=0),
                )
                nc.vector.tensor_add(out=g0[:], in0=g0[:], in1=g1[:])
                nc.sync.dma_start(out[:, i], g0[:])
    nc.compile()
    return nc


# ---------------------------------------------------------------- packing
def _try_assign(units, tpl_u, slack_cap):
    """Cover each expert's unit count with slots from 8x tpl_u inventory.

    units/tpl_u are in 128-token units. slack_cap bounds per-expert
    over-allocation (in units). Returns {expert: {size: n}} or None.
    """
    inv = {}
    for s in tpl_u:
        inv[s] = inv.get(s, 0) + NCORES
    sizes = sorted(inv, reverse=True)
    experts = sorted(range(len(units)), key=lambda e: -units[e])
    budget = [0]

    def covers(c, i, slack, out, cur):
        if len(out) >= 40 or budget[0] > 100000:
            return
        budget[0] += 1
        if c <= 0:
            if -c <= slack:
                out.append(dict(cur))
            return
        if i >= len(sizes):
            return
        s = sizes[i]
        hi = min(inv[s], (c + slack) // s)
        for n in range(hi, -1, -1):
            if i == len(sizes) - 1 and n * s < c:
                break  # last size can't cover the remainder
            if n:
                cur[s] = n
            covers(c - n * s, i + 1, slack, out, cur)
            cur.pop(s, None)

    def dfs(idx):
        budget[0] += 1
        if budget[0] > 100000:
            return None
        if idx == len(experts):
            return {}
        e = experts[idx]
        c = int(units[e])
        if c == 0:
            rest = dfs(idx + 1)
            if rest is not None:
                rest[e] = {}
            return rest
        out = []
        covers(c, 0, slack_cap, out, {})
        out.sort(key=lambda d: (sum(s * n for s, n in d.items()) - c, sum(d.values())))
        for cov in out[:16]:
            if not all(inv[s] >= n for s, n in cov.items()):
                continue
            for s, n in cov.items():
                inv[s] -= n
            rest = dfs(idx + 1)
            if rest is not None:
                rest[e] = cov
                return rest
            for s, n in cov.items():
                inv[s] += n
        return None

    return dfs(0)


def _solve_packing(counts):
    """Pick per-core slot-size template (uniform across cores) + expert cover.

    Works in 128-token units; returns (template_in_tokens, assign) where
    assign maps expert -> {slot_size_tokens: n_slots}.
    """
    G = 32  # slot-size granularity in tokens
    units = [-(-int(c) // G) for c in counts]
    U0 = max(1, -(-sum(units) // NCORES))
    maxu = max(units + [1])

    def partitions(total, maxpart, maxparts):
        if total == 0:
            yield ()
            return
        if maxparts == 0:
            return
        for p in range(min(total, maxpart), 0, -1):
            for rest in partitions(total - p, p, maxparts - 1):
                yield (p,) + rest

    for U in range(U0, U0 + 24):
        tpls = sorted(
            {t for t in partitions(U, 1024 // G, 5)},
            key=lambda t: (len(t), -min(t), tuple(-v for v in t)),
        )[:64]
        for slack in (0, 1, 2, 4):
            for tpl_u in tpls:
                asg = _try_assign(units, tpl_u, slack)
                if asg is not None:
                    tpl = tuple(s * G for s in tpl_u)
                    return tpl, {
                        e: {s * G: n for s, n in cov.items()}
                        for e, cov in asg.items()
                    }
    # retry coarser granularity before the uniform fallback
    for G2 in (64, 128):
        U0b = max(1, -(-sum(-(-int(c) // G2) for c in counts) // NCORES))
        for U in range(U0b, U0b + 24):
            tpls = sorted(
                {t for t in partitions(U, 1024 // G2, 5)},
                key=lambda t: (len(t), -min(t), tuple(-v for v in t)),
            )[:64]
            for slack in (0, 1, 2, 4):
                for tpl_u in tpls:
                    asg = _try_assign([-(-int(c) // G2) for c in counts], tpl_u, slack)
                    if asg is not None:
                        return tuple(s * G2 for s in tpl_u), {
                            e: {s * G2: n for s, n in cov.items()}
                            for e, cov in asg.items()
                        }
    # ultimate fallback: uniform 2 slots covering the largest expert
    S = max(128, (int(max(counts)) // 2 // 128 + 1) * 128)
    while True:
        if sum(-(-int(c) // S) for c in counts if c) <= 2 * NCORES:
            asg = {e: {S: -(-int(c) // S)} for e, c in enumerate(counts) if c}
            return (S, S), asg
        S += 128
    raise RuntimeError("no packing found")


# ---------------------------------------------------------------- kernel
def kernel(x, W1, b1, W2, b2, W3, b3, Wg1, bg1, Wg2, bg2, top_k):
    x = np.asarray(x, np.float32)
    W1 = np.asarray(W1, np.float32)
    W2 = np.asarray(W2, np.float32)
    W3 = np.asarray(W3, np.float32)
    Wg1 = np.asarray(Wg1, np.float32)
    Wg2 = np.asarray(Wg2, np.float32)
    assert int(np.asarray(top_k)) == 2
    for b in (b1, b2, b3, bg1, bg2):
        assert not np.any(np.asarray(b)), "nonzero biases unsupported"

    core_ids = list(range(NCORES))
    NT = TPC // 128

    # ---------------- gate: logits + softmax on device ----------------
    if "gate" not in _CACHE:
        _CACHE["gate"] = _build_gate_nc()
    xT32 = np.ascontiguousarray(x.T)  # [D, N] fp32
    xT = xT32.astype(BF)  # bf16 hi part, also the expert-MLP input
    xTlo = (xT32 - xT.astype(np.float32)).astype(BF)  # bf16 residual
    wg1p = np.zeros((D, 128), np.float32)
    wg1p[:, :64] = Wg1
    wg2p = np.zeros((128, 128), np.float32)
    wg2p[:64, :E] = Wg2
    wg1hi = _pmn(wg1p)
    wg1lo = _pmn(wg1p - wg1hi.transpose(1, 0, 2).reshape(D, 128).astype(np.float32))

    def _ptile(a, c):  # [D, N] -> per-core [128, D/128, TPC]
        return np.ascontiguousarray(
            a[:, c * TPC : (c + 1) * TPC].reshape(D // 128, 128, TPC).transpose(1, 0, 2)
        )

    in1 = [
        {
            "xhi": _ptile(xT, c),
            "xlo": _ptile(xTlo, c),
            "wg1hi": wg1hi,
            "wg1lo": wg1lo,
            "wg2": wg2p,
        }
        for c in core_ids
    ]
    res1 = run_bass_kernel_spmd(_CACHE["gate"], in1, core_ids).results
    # probs[p, i, e] -> token c*TPC + i*128 + p
    probs = np.concatenate(
        [res1[c]["probs"].transpose(1, 0, 2).reshape(TPC, E) for c in core_ids], axis=0
    ).astype(np.float32)  # [N, E]

    # ---------------- host routing (comparisons/indexing only) ----------------
    top2 = np.argsort(-probs, axis=1, kind="stable")[:, :2]  # [N, 2]
    e0s, e1s = top2[:, 0], top2[:, 1]
    expert_lists = [np.nonzero((top2 == e).any(axis=1))[0] for e in range(E)]
    counts = np.array([len(t) for t in expert_lists])

    tpl, asg = _solve_packing(counts)
    C = sum(tpl)

    # slot positions per size: (core, slot_idx) pools
    pos_pool = {}
    for jj, s in enumerate(tpl):
        pos_pool.setdefault(s, []).extend((c, jj) for c in core_ids)
    slot_off = {}  # (core, j) -> global row offset in yall
    for c in core_ids:
        off = c * C
        for jj, s in enumerate(tpl):
            slot_off[(c, jj)] = off
            off += s
    # assign slots to experts, chop token lists
    slot_map = {}  # (core, j) -> (expert, token_array)
    glob_row = np.zeros((N, E), np.int64)
    for e in range(E):
        tl = expert_lists[e]
        claims = []
        for s in sorted(asg.get(e, {}), reverse=True):
            for _ in range(asg[e][s]):
                claims.append((s, pos_pool[s].pop()))
        cum = 0
        for s, posn in claims:
            chunk = tl[cum : cum + s]
            slot_map[posn] = (e, chunk)
            glob_row[chunk, e] = slot_off[posn] + np.arange(len(chunk))
            cum += s
        assert cum >= len(tl), f"expert {e} not fully covered"
    # leftover slots empty
    for s, pool in pos_pool.items():
        for posn in pool:
            slot_map[posn] = (0, np.zeros(0, np.int64))

    w_tok = np.take_along_axis(probs, top2, axis=1)  # [N, 2] combine weights

    # ---------------- mlp: fused expert MLP on device ----------------
    key2 = (("emlp8" if USE_FP8 else "emlp"), tpl)
    if key2 not in _CACHE:
        _CACHE[key2] = _build_mlp_fp8_nc(tpl) if USE_FP8 else _build_mlp_nc(tpl)
    Wb = {}

    def _mtile(a):
        # [128, K/128, M] -> [M/128, 128, K/128, 128] (m-tile contiguous)
        P, KT, M = a.shape
        return np.ascontiguousarray(
            a.reshape(P, KT, M // 128, 128).transpose(2, 0, 1, 3)
        )

    def _split8(W, sw):
        Ws = W * sw
        hi = Ws.astype(E4M3)
        lo = (Ws - hi.astype(np.float32)).astype(E4M3)
        # concat hi and lo along the k-tile axis: [MT, 128, 2*KT, 128]
        return np.ascontiguousarray(
            np.concatenate([_mtile(_pmn(hi, E4M3)), _mtile(_pmn(lo, E4M3))], axis=2)
        )

    def wts(e):
        if e not in Wb:
            if USE_FP8:
                Wb[e] = (
                    _split8(W1[e], SW1),
                    _split8(W2[e], SW2),
                    _split8(W3[e], SW3),
                )
            else:
                Wb[e] = (_mtile(_pmn(W1[e])), _mtile(_pmn(W2[e])), _mtile(_pmn(W3[e])))
        return Wb[e]

    if USE_FP8:
        Xs = xT32 * SX
        xhi8 = Xs.astype(E4M3)
        xlo8 = (Xs - xhi8.astype(np.float32)).astype(E4M3)

    wmask = np.zeros((N, E), np.float32)
    wmask[np.arange(N), e0s] = w_tok[:, 0]
    wmask[np.arange(N), e1s] = w_tok[:, 1]

    def _pt(a, S):  # [D, S] -> [128, D/128, S]
        return np.ascontiguousarray(a.reshape(D // 128, 128, S).transpose(1, 0, 2))

    in2 = []
    for c in core_ids:
        d = {}
        for jj, S in enumerate(tpl):
            e, chunk = slot_map[(c, jj)]
            padded = np.zeros(S, np.int64)
            padded[: len(chunk)] = chunk
            wv = np.zeros(S, np.float32)
            wv[: len(chunk)] = wmask[chunk, e]
            if USE_FP8:
                d[f"xhi{jj}"] = _pt(xhi8[:, padded], S)
                d[f"xlo{jj}"] = _pt(xlo8[:, padded], S)
                d[f"w1_{jj}"], d[f"w2_{jj}"], d[f"w3_{jj}"] = wts(e)
                wv = wv / (SW3 * SH)
            else:
                w1p, w2p, w3p = wts(e)
                d[f"xt{jj}"] = _pt(xT[:, padded], S)
                d[f"w1_{jj}"] = w1p
                d[f"w2_{jj}"] = w2p
                d[f"w3_{jj}"] = w3p
            d[f"wv{jj}"] = np.ascontiguousarray(
                np.broadcast_to(wv[None, :], (128, S))
            )
        in2.append(d)
    res2 = run_bass_kernel_spmd(_CACHE[key2], in2, core_ids).results

    R = NCORES * C
    yall = np.empty((R, O), BF)
    for c in core_ids:
        for jj, S in enumerate(tpl):
            yt = res2[c][f"yt{jj}"]  # [128, O/128, S]
            off = slot_off[(c, jj)]
            yall[off : off + S] = yt.transpose(2, 1, 0).reshape(S, O)

    # ---------------- combine: gather 2 scaled rows + add ----------------
    key3 = ("comb", R)
    if key3 not in _CACHE:
        _CACHE[key3] = _build_comb_nc(R)
    g0 = glob_row[np.arange(N), e0s].astype(np.int32)
    g1 = glob_row[np.arange(N), e1s].astype(np.int32)
    in3 = []
    for c in core_ids:
        sl = slice(c * TPC, (c + 1) * TPC)
        in3.append(
            {
                "yall": yall,
                "i0": np.ascontiguousarray(g0[sl].reshape(NT, 128).T),
                "i1": np.ascontiguousarray(g1[sl].reshape(NT, 128).T),
            }
        )
    res3 = run_bass_kernel_spmd(_CACHE[key3], in3, core_ids).results
    out = np.concatenate(
        [
            res3[c]["out"].transpose(1, 0, 2).reshape(TPC, O).astype(np.float32)
            for c in core_ids
        ],
        axis=0,
    )
    return out


# revision 33
# speedup vs baseline: 1.0138x; 1.0019x over previous
"""MoE (8 experts, top-2) Trainium2 Bass kernel, expert-parallel over 8 cores.

Pipeline (all FLOPs on device):
  gate   : gating logits + softmax for all tokens (data-parallel over cores),
           output token-major probs [N, E]
  host   : top-2 selection + slot packing (comparisons/indexing only)
  mlp    : per-core fused 3-layer expert MLP on gathered tokens (bf16 matmuls,
           h1/h2 resident in SBUF, per-token gate weight folded into L3 evict)
  combine: per-token gather of its two scaled expert rows + add
"""

import numpy as np
import ml_dtypes

import jax

jax.config.update("jax_compilation_cache_dir", "/tmp/jax_comp_cache")
jax.config.update("jax_persistent_cache_min_entry_size_bytes", -1)
jax.config.update("jax_persistent_cache_min_compile_time_secs", 0)

import concourse.bass as bass
import concourse.mybir as mybir
import concourse.tile as tile
from concourse import bacc
from concourse.bass_utils import run_bass_kernel_spmd

N, D, H, O, E = 8192, 1024, 2048, 1024, 8
NCORES = 8
TPC = N // NCORES  # tokens per core for gate/combine phases
F32 = mybir.dt.float32
BF16 = mybir.dt.bfloat16
I32 = mybir.dt.int32
BF = ml_dtypes.bfloat16
ACT = mybir.ActivationFunctionType

_CACHE = {}


def _nc():
    return bacc.Bacc(None, target_bir_lowering=False, debug=True)


def _pmn(a, dtype=BF):
    """[K, M] row-major -> [128, K/128, M] with row k = m*128 + p."""
    K, M = a.shape
    return np.ascontiguousarray(
        a.reshape(K // 128, 128, M).transpose(1, 0, 2).astype(dtype)
    )


# ---------------------------------------------------------------- gate
def _build_gate_nc():
    # Logits need fp32-class accuracy so top-2 selection and combine weights
    # match the fp32 reference on near-tied gates (bf16 logits flip ~0.6% of
    # tokens' second expert -> 5e-2 output error). L1 uses a 3-term bf16
    # residual split (err ~2^-17), L2 runs in true fp32 (4 cyc/row, tiny).
    nc = _nc()
    NT = TPC // 128  # token tiles per core
    xhi = nc.dram_tensor("xhi", [128, D // 128, TPC], BF16, kind="ExternalInput")
    xlo = nc.dram_tensor("xlo", [128, D // 128, TPC], BF16, kind="ExternalInput")
    wg1hi = nc.dram_tensor("wg1hi", [128, D // 128, 128], BF16, kind="ExternalInput")
    wg1lo = nc.dram_tensor("wg1lo", [128, D // 128, 128], BF16, kind="ExternalInput")
    wg2 = nc.dram_tensor("wg2", [128, 128], F32, kind="ExternalInput")
    probs = nc.dram_tensor("probs", [128, NT, E], F32, kind="ExternalOutput")
    X = mybir.AxisListType.X
    with tile.TileContext(nc) as tc:
        with (
            tc.tile_pool(name="w", bufs=1) as wp,
            tc.tile_pool(name="x", bufs=2) as xp,
            tc.tile_pool(name="g", bufs=1) as gp,
            tc.tile_pool(name="s", bufs=2) as sp,
            tc.tile_pool(name="ps", bufs=3, space="PSUM") as pp,
            tc.tile_pool(name="warm", bufs=1, space="PSUM") as wmp,
        ):
            whi = wp.tile([128, D // 128, 128], BF16)
            wlo = wp.tile([128, D // 128, 128], BF16)
            wg2t = wp.tile([128, 128], F32)
            nc.sync.dma_start(wg2t[:], wg2[:])
            nc.sync.dma_start(whi[:], wg1hi[:])
            nc.sync.dma_start(wlo[:], wg1lo[:])
            # PE p-state warmup: the cost model ramps 0.65->1.2->2.4 GHz over
            # 3us of continuous PE activity; dummy matmuls on the earliest-
            # arriving tile (wg2, 64KB) keep the clock ramping while x streams.
            warm = wmp.tile([128, 128], F32, tag="warm")
            for _ in range(16):
                nc.tensor.matmul(warm[:], lhsT=wg2t[:], rhs=wg2t[:], start=True, stop=True)
            # L1: g1 = relu(Wg1^T x) ~= relu((Whi+Wlo)^T xhi + Whi^T xlo)
            # x loaded per 512-token chunk so DMA pipelines with compute
            g1 = gp.tile([128, NT // 4, 512], F32)  # [feat, chunk, token]
            for ci, i in enumerate(range(0, TPC, 512)):
                xh = xp.tile([128, D // 128, 512], BF16, tag="xh", name="xh")
                xl = xp.tile([128, D // 128, 512], BF16, tag="xl", name="xl")
                nc.sync.dma_start(xh[:], xhi[:, :, i : i + 512])
                nc.sync.dma_start(xl[:], xlo[:, :, i : i + 512])
                ps = pp.tile([128, 512], F32, tag="ps1")
                terms = [(whi, xh), (wlo, xh), (whi, xl)]
                for ti, (wt, xt_) in enumerate(terms):
                    for k in range(D // 128):
                        nc.tensor.matmul(
                            ps[:],
                            lhsT=wt[:, k],
                            rhs=xt_[:, k],
                            start=(ti == 0 and k == 0),
                            stop=(ti == 2 and k == D // 128 - 1),
                        )
                nc.scalar.activation(g1[:, ci], ps[:], ACT.Relu)
            # L2 (token-major, fp32): logitsT[t, e] = sum_f g1[f, t] Wg2[f, e]
            ex = sp.tile([128, NT, E], F32, tag="ex")
            for i in range(NT):
                ps = pp.tile([128, 128], F32, tag="ps2")
                nc.tensor.matmul(
                    ps[:],
                    lhsT=g1[:, i // 4, (i % 4) * 128 : (i % 4 + 1) * 128],
                    rhs=wg2t[:],
                    start=True,
                    stop=True,
                )
                nc.scalar.activation(ex[:, i], ps[:, :E], ACT.Exp)
            # softmax + output in two halves: the first half's chain runs
            # while L2 tiles 4-7 are still on the PE, shortening the tail
            for hh in range(2):
                exh = ex[:, hh * 4 : hh * 4 + 4]
                s = sp.tile([128, 4, 1], F32, tag="s", name="s")
                nc.vector.reduce_sum(out=s[:], in_=exh, axis=X)
                inv = sp.tile([128, 4, 1], F32, tag="inv", name="inv")
                nc.vector.reciprocal(out=inv[:], in_=s[:])
                pr = sp.tile([128, 4, E], F32, tag="pr", name="pr")
                nc.vector.tensor_mul(out=pr[:], in0=exh, in1=inv[:].to_broadcast((128, 4, E)))
                nc.sync.dma_start(probs[:, hh * 4 : hh * 4 + 4], pr[:])
    nc.compile()
    return nc


# ---------------------------------------------------------------- mlp
def _build_mlp_nc(tpl):
    """Fused 3-layer expert MLP; one slot per template entry, bf16 matmuls.

    Slot j: xt{j} [128, D/128, S] bf16, w1_{j} [128, D/128, H], w2_{j}
    [128, H/128, H], w3_{j} [128, H/128, O] bf16, wv{j} [128, S] f32
    (per-token combine weight, pre-broadcast), output yt{j} [128, O/128, S]
    bf16 (already scaled by wv).
    """
    nc = _nc()
    t = {}
    for j, S in enumerate(tpl):
        # weights pre-tiled host-side: [m][p, k, col] so each m-tile DMA is
        # one contiguous 2KB-per-partition transfer (no small-descriptor
        # penalty)
        t[f"xt{j}"] = nc.dram_tensor(f"xt{j}", [128, D // 128, S], BF16, kind="ExternalInput")
        t[f"w1_{j}"] = nc.dram_tensor(f"w1_{j}", [H // 128, 128, D // 128, 128], BF16, kind="ExternalInput")
        t[f"w2_{j}"] = nc.dram_tensor(f"w2_{j}", [H // 128, 128, H // 128, 128], BF16, kind="ExternalInput")
        t[f"w3_{j}"] = nc.dram_tensor(f"w3_{j}", [O // 128, 128, H // 128, 128], BF16, kind="ExternalInput")
        t[f"wv{j}"] = nc.dram_tensor(f"wv{j}", [128, S], F32, kind="ExternalInput")
        t[f"yt{j}"] = nc.dram_tensor(f"yt{j}", [128, O // 128, S], BF16, kind="ExternalOutput")
    Smax = max(tpl)
    with tile.TileContext(nc) as tc:
        with (
            tc.tile_pool(name="x", bufs=2) as xp,
            tc.tile_pool(name="w1", bufs=3) as w1p,
            tc.tile_pool(name="w2", bufs=3) as w2p,
            tc.tile_pool(name="w3", bufs=3) as w3p,
            tc.tile_pool(name="h", bufs=2) as hp,
            tc.tile_pool(name="wv", bufs=2) as wvp,
            tc.tile_pool(name="y", bufs=4) as yp,
            tc.tile_pool(name="ps", bufs=6, space="PSUM") as pp,
        ):
            for j, S in enumerate(tpl):
                chunks = [(o, min(512, S - o)) for o in range(0, S, 512)]
                xsb = xp.tile([128, D // 128, Smax], BF16, tag="x")
                for k in range(D // 128):
                    nc.sync.dma_start(xsb[:, k, :S], t[f"xt{j}"][:, k])
                wv = wvp.tile([128, Smax], F32, tag="wv")
                nc.sync.dma_start(wv[:, :S], t[f"wv{j}"][:])
                h1 = hp.tile([128, H // 128, Smax], BF16, tag="h1")
                for m in range(H // 128):
                    w = w1p.tile([128, D // 128, 128], BF16, tag="w1")
                    nc.sync.dma_start(w[:], t[f"w1_{j}"][m])
                    for o, sz in chunks:
                        ps = pp.tile([128, 512], F32, tag="ps", name="ps")[:, :sz]
                        for k in range(D // 128):
                            nc.tensor.matmul(
                                ps,
                                lhsT=w[:, k],
                                rhs=xsb[:, k, o : o + sz],
                                start=(k == 0),
                                stop=(k == D // 128 - 1),
                            )
                        nc.scalar.activation(h1[:, m, o : o + sz], ps, ACT.Relu)
                h2 = hp.tile([128, H // 128, Smax], BF16, tag="h2")
                for m in range(H // 128):
                    w = w2p.tile([128, H // 128, 128], BF16, tag="w2")
                    nc.sync.dma_start(w[:], t[f"w2_{j}"][m])
                    for o, sz in chunks:
                        ps = pp.tile([128, 512], F32, tag="ps", name="ps")[:, :sz]
                        for k in range(H // 128):
                            nc.tensor.matmul(
                                ps,
                                lhsT=w[:, k],
                                rhs=h1[:, k, o : o + sz],
                                start=(k == 0),
                                stop=(k == H // 128 - 1),
                            )
                        nc.scalar.activation(h2[:, m, o : o + sz], ps, ACT.Relu)
                for m in range(O // 128):
                    w = w3p.tile([128, H // 128, 128], BF16, tag="w3")
                    nc.sync.dma_start(w[:], t[f"w3_{j}"][m])
                    for o, sz in chunks:
                        ps = pp.tile([128, 512], F32, tag="ps", name="ps")[:, :sz]
                        for k in range(H // 128):
                            nc.tensor.matmul(
                                ps,
                                lhsT=w[:, k],
                                rhs=h2[:, k, o : o + sz],
                                start=(k == 0),
                                stop=(k == H // 128 - 1),
                            )
                        y = yp.tile([128, 512], BF16, tag="y", name="y")[:, :sz]
                        nc.vector.tensor_mul(out=y, in0=ps, in1=wv[:, o : o + sz])
                        nc.sync.dma_start(t[f"yt{j}"][:, m, o : o + sz], y)
    nc.compile()
    return nc


# ---------------------------------------------------------------- fp8 mlp
# 3-term residual-split fp8 matmuls in DoubleRow perf mode (2 contraction
# rows per PE pass): W ~= (Whi + Wlo)/sw, x ~= (xhi + xlo)/sx, and
# W^T x ~= (Whi^T xhi + Wlo^T xhi + Whi^T xlo) / (sw sx), dropping only the
# lo*lo term (~1e-3 relative). Measured end-to-end rel err ~3.4e-3.
FP8 = mybir.dt.float8e4
E4M3 = ml_dtypes.float8_e4m3
SX, SH = 32.0, 32.0
SW1, SW2, SW3 = 512.0, 1024.0, 1024.0
USE_FP8 = True


def _build_mlp_fp8_nc(tpl):
    nc = _nc()
    t = {}
    KD, KH = D // 128, H // 128
    for j, S in enumerate(tpl):
        t[f"xhi{j}"] = nc.dram_tensor(f"xhi{j}", [128, KD, S], FP8, kind="ExternalInput")
        t[f"xlo{j}"] = nc.dram_tensor(f"xlo{j}", [128, KD, S], FP8, kind="ExternalInput")
        for L, KT, MT in (("1", KD, KH), ("2", KH, KH), ("3", KH, O // 128)):
            # hi and lo halves concatenated along k so one DMA loads both
            # (DMA issue costs ~650ns of SEQ+HWDGE each; count matters)
            t[f"w{L}_{j}"] = nc.dram_tensor(
                f"w{L}_{j}", [MT, 128, 2 * KT, 128], FP8, kind="ExternalInput"
            )
        t[f"wv{j}"] = nc.dram_tensor(f"wv{j}", [128, S], F32, kind="ExternalInput")
        t[f"yt{j}"] = nc.dram_tensor(f"yt{j}", [128, O // 128, S], BF16, kind="ExternalOutput")
    Smax = max(tpl)
    DR = mybir.MatmulPerfMode.DoubleRow

    with tile.TileContext(nc) as tc:
        with (
            tc.tile_pool(name="x", bufs=2) as xp,
            tc.tile_pool(name="w1", bufs=4) as w1p,
            tc.tile_pool(name="w2", bufs=6) as w2p,
            tc.tile_pool(name="w3", bufs=4) as w3p,
            tc.tile_pool(name="h", bufs=1) as hp,
            tc.tile_pool(name="hb", bufs=4) as hbp,
            tc.tile_pool(name="wv", bufs=2) as wvp,
            tc.tile_pool(name="y", bufs=3) as yp,
            tc.tile_pool(name="ps", bufs=6, space="PSUM") as pp,
            tc.tile_pool(name="warm", bufs=1, space="PSUM") as wmp,
        ):
            warm = wmp.tile([128, 128], F32, tag="warm")
            for j, S in enumerate(tpl):
                chunks = [(o, min(512, S - o)) for o in range(0, S, 512)]
                # split x loads so the first matmul group's operands (k-tiles
                # 0..1 of xhi then xlo) land first; wv is deferred to L3
                xh = xp.tile([128, KD, Smax], FP8, tag="xh")
                xl = xp.tile([128, KD, Smax], FP8, tag="xl")
                nc.sync.dma_start(xh[:, :2, :S], t[f"xhi{j}"][:, :2])
                nc.sync.dma_start(xh[:, 2:, :S], t[f"xhi{j}"][:, 2:])
                nc.sync.dma_start(xl[:, :2, :S], t[f"xlo{j}"][:, :2])
                nc.sync.dma_start(xl[:, 2:, :S], t[f"xlo{j}"][:, 2:])
                if j == 0:
                    # PE p-state warmup on the first-arriving x tile
                    for _ in range(28):
                        nc.tensor.matmul(
                            warm[:], lhsT=xh[:, 0, :128], rhs=xh[:, 0, :128],
                            start=True, stop=True,
                        )
                wv = wvp.tile([128, Smax], F32, tag="wv")

                def layer(L, KT, MT, ihi, ilo, wpool, out_cb):
                    for m in range(MT):
                        w = wpool.tile([128, 2 * KT, 128], FP8, tag=f"w{L}")
                        nc.sync.dma_start(w[:], t[f"w{L}_{j}"][m])
                        whi, wlo = w[:, :KT], w[:, KT:]
                        terms = [(whi, ihi), (wlo, ihi), (whi, ilo)]
                        for o, sz in chunks:
                            ps = pp.tile([128, 512], F32, tag="ps", name="ps")[:, :sz]
                            for ti, (wt, it) in enumerate(terms):
                                for k2 in range(KT // 2):
                                    nc.tensor.matmul(
                                        ps,
                                        lhsT=wt[:, 2 * k2 : 2 * k2 + 2, :],
                                        rhs=it[:, 2 * k2 : 2 * k2 + 2, o : o + sz],
                                        start=(ti == 0 and k2 == 0),
                                        stop=(ti == 2 and k2 == KT // 2 - 1),
                                        perf_mode=DR,
                                    )
                            out_cb(m, o, sz, ps)

                h1hi = hp.tile([128, KH, Smax], FP8, tag="h1hi")
                h1lo = hp.tile([128, KH, Smax], FP8, tag="h1lo")

                def evict1(m, o, sz, ps):
                    hb = hbp.tile([128, 512], BF16, tag="hb", name="hb")[:, :sz]
                    nc.scalar.activation(hb, ps, ACT.Relu, scale=SH / (SW1 * SX))
                    nc.scalar.activation(h1hi[:, m, o : o + sz], hb, ACT.Copy)
                    nc.vector.tensor_tensor(
                        h1lo[:, m, o : o + sz], hb, h1hi[:, m, o : o + sz],
                        mybir.AluOpType.subtract,
                    )

                layer("1", KD, KH, xh, xl, w1p, evict1)

                h2hi = hp.tile([128, KH, Smax], FP8, tag="h2hi")
                h2lo = hp.tile([128, KH, Smax], FP8, tag="h2lo")

                def evict2(m, o, sz, ps):
                    hb = hbp.tile([128, 512], BF16, tag="hb", name="hb")[:, :sz]
                    nc.scalar.activation(hb, ps, ACT.Relu, scale=1.0 / SW2)
                    nc.scalar.activation(h2hi[:, m, o : o + sz], hb, ACT.Copy)
                    nc.vector.tensor_tensor(
                        h2lo[:, m, o : o + sz], hb, h2hi[:, m, o : o + sz],
                        mybir.AluOpType.subtract,
                    )

                layer("2", KH, KH, h1hi, h1lo, w2p, evict2)

                nc.sync.dma_start(wv[:, :S], t[f"wv{j}"][:])
                ycur = [None]

                def evict3(m, o, sz, ps):
                    if ycur[0] is None:
                        ycur[0] = yp.tile([128, Smax], BF16, tag="y", name="y")
                    nc.vector.tensor_mul(
                        out=ycur[0][:, o : o + sz], in0=ps, in1=wv[:, o : o + sz]
                    )
                    if o + sz >= S:  # last chunk of this m: flush one DMA
                        nc.sync.dma_start(t[f"yt{j}"][:, m, :S], ycur[0][:, :S])
                        ycur[0] = None

                layer("3", KH, O // 128, h2hi, h2lo, w3p, evict3)
    nc.compile()
    return nc


# ---------------------------------------------------------------- combine
def _build_comb_nc(R):
    nc = _nc()
    NT = TPC // 128
    yall = nc.dram_tensor("yall", [R, O], BF16, kind="ExternalInput")
    i0 = nc.dram_tensor("i0", [128, NT], I32, kind="ExternalInput")
    i1 = nc.dram_tensor("i1", [128, NT], I32, kind="ExternalInput")
    out = nc.dram_tensor("out", [128, NT, O], BF16, kind="ExternalOutput")
    with tile.TileContext(nc) as tc:
        with (
            tc.tile_pool(name="big", bufs=6) as gp,
            tc.tile_pool(name="idx", bufs=1) as ip,
        ):
            i0_t = ip.tile([128, NT], I32)
            i1_t = ip.tile([128, NT], I32)
            nc.sync.dma_start(i0_t[:], i0[:])
            nc.sync.dma_start(i1_t[:], i1[:])
            # per-tile single-index gathers (the runtime only supports one
            # index per partition per indirect DMA). Independent g0/g1 tiles
            # keep the Pool SWDGE queue saturated; DVE adds and per-tile
            # output writes pipeline underneath.
            for i in range(NT):
                g0 = gp.tile([128, O], BF16, tag="g0", name="g0")
                g1 = gp.tile([128, O], BF16, tag="g1", name="g1")
                nc.gpsimd.indirect_dma_start(
                    out=g0[:],
                    out_offset=None,
                    in_=yall[:],
                    in_offset=bass.IndirectOffsetOnAxis(ap=i0_t[:, i : i + 1], axis=0),
                )
                nc.gpsimd.indirect_dma_start(
                    out=g1[:],
                    out_offset=None,
                    in_=yall[:],
                    in_offset=bass.IndirectOffsetOnAxis(ap=i1_t[:, i : i + 1], axis=0),
                )
                nc.vector.tensor_add(out=g0[:], in0=g0[:], in1=g1[:])
                nc.sync.dma_start(out[:, i], g0[:])
    nc.compile()
    return nc


# ---------------------------------------------------------------- packing
def _try_assign(units, tpl_u, slack_cap):
    """Cover each expert's unit count with slots from 8x tpl_u inventory.

    units/tpl_u are in 128-token units. slack_cap bounds per-expert
    over-allocation (in units). Returns {expert: {size: n}} or None.
    """
    inv = {}
    for s in tpl_u:
        inv[s] = inv.get(s, 0) + NCORES
    sizes = sorted(inv, reverse=True)
    experts = sorted(range(len(units)), key=lambda e: -units[e])
    budget = [0]

    def covers(c, i, slack, out, cur):
        if len(out) >= 40 or budget[0] > 100000:
            return
        budget[0] += 1
        if c <= 0:
            if -c <= slack:
                out.append(dict(cur))
            return
        if i >= len(sizes):
            return
        s = sizes[i]
        hi = min(inv[s], (c + slack) // s)
        for n in range(hi, -1, -1):
            if i == len(sizes) - 1 and n * s < c:
                break  # last size can't cover the remainder
            if n:
                cur[s] = n
            covers(c - n * s, i + 1, slack, out, cur)
            cur.pop(s, None)

    def dfs(idx):
        budget[0] += 1
        if budget[0] > 100000:
            return None
        if idx == len(experts):
            return {}
        e = experts[idx]
        c = int(units[e])
        if c == 0:
            rest = dfs(idx + 1)
            if rest is not None:
                rest[e] = {}
            return rest
        out = []
        covers(c, 0, slack_cap, out, {})
        out.sort(key=lambda d: (sum(s * n for s, n in d.items()) - c, sum(d.values())))
        for cov in out[:16]:
            if not all(inv[s] >= n for s, n in cov.items()):
                continue
            for s, n in cov.items():
                inv[s] -= n
            rest = dfs(idx + 1)
            if rest is not None:
                rest[e] = cov
                return rest
            for s, n in cov.items():
                inv[s] += n
        return None

    return dfs(0)


def _solve_packing(counts):
    """Pick per-core slot-size template (uniform across cores) + expert cover.

    Works in 128-token units; returns (template_in_tokens, assign) where
    assign maps expert -> {slot_size_tokens: n_slots}.
    """
    G = 32  # slot-size granularity in tokens
    units = [-(-int(c) // G) for c in counts]
    U0 = max(1, -(-sum(units) // NCORES))
    maxu = max(units + [1])

    def partitions(total, maxpart, maxparts):
        if total == 0:
            yield ()
            return
        if maxparts == 0:
            return
        for p in range(min(total, maxpart), 0, -1):
            for rest in partitions(total - p, p, maxparts - 1):
                yield (p,) + rest

    for U in range(U0, U0 + 24):
        tpls = sorted(
            {t for t in partitions(U, 1024 // G, 5)},
            key=lambda t: (len(t), -min(t), tuple(-v for v in t)),
        )[:64]
        for slack in (0, 1, 2, 4):
            for tpl_u in tpls:
                asg = _try_assign(units, tpl_u, slack)
                if asg is not None:
                    tpl = tuple(s * G for s in tpl_u)
                    return tpl, {
                        e: {s * G: n for s, n in cov.items()}
                        for e, cov in asg.items()
                    }
    # retry coarser granularity before the uniform fallback
    for G2 in (64, 128):
        U0b = max(1, -(-sum(-(-int(c) // G2) for c in counts) // NCORES))
        for U in range(U0b, U0b + 24):
            tpls = sorted(
                {t for t in partitions(U, 1024 // G2, 5)},
                key=lambda t: (len(t), -min(t), tuple(-v for v in t)),
            )[:64]
            for slack in (0, 1, 2, 4):
                for tpl_u in tpls:
                    asg = _try_assign([-(-int(c) // G2) for c in counts], tpl_u, slack)
                    if asg is not None:
                        return tuple(s * G2 for s in tpl_u), {
                            e: {s * G2: n for s, n in cov.items()}
                            for e, cov in asg.items()
                        }
    # ultimate fallback: uniform 2 slots covering the largest expert
    S = max(128, (int(max(counts)) // 2 // 128 + 1) * 128)
    while True:
        if sum(-(-int(c) // S) for c in counts if c) <= 2 * NCORES:
            asg = {e: {S: -(-int(c) // S)} for e, c in enumerate(counts) if c}
            return (S, S), asg
        S += 128
    raise RuntimeError("no packing found")


# ---------------------------------------------------------------- kernel
def kernel(x, W1, b1, W2, b2, W3, b3, Wg1, bg1, Wg2, bg2, top_k):
    x = np.asarray(x, np.float32)
    W1 = np.asarray(W1, np.float32)
    W2 = np.asarray(W2, np.float32)
    W3 = np.asarray(W3, np.float32)
    Wg1 = np.asarray(Wg1, np.float32)
    Wg2 = np.asarray(Wg2, np.float32)
    assert int(np.asarray(top_k)) == 2
    for b in (b1, b2, b3, bg1, bg2):
        assert not np.any(np.asarray(b)), "nonzero biases unsupported"

    core_ids = list(range(NCORES))
    NT = TPC // 128

    # ---------------- gate: logits + softmax on device ----------------
    if "gate" not in _CACHE:
        _CACHE["gate"] = _build_gate_nc()
    xT32 = np.ascontiguousarray(x.T)  # [D, N] fp32
    xT = xT32.astype(BF)  # bf16 hi part, also the expert-MLP input
    xTlo = (xT32 - xT.astype(np.float32)).astype(BF)  # bf16 residual
    wg1p = np.zeros((D, 128), np.float32)
    wg1p[:, :64] = Wg1
    wg2p = np.zeros((128, 128), np.float32)
    wg2p[:64, :E] = Wg2
    wg1hi = _pmn(wg1p)
    wg1lo = _pmn(wg1p - wg1hi.transpose(1, 0, 2).reshape(D, 128).astype(np.float32))

    def _ptile(a, c):  # [D, N] -> per-core [128, D/128, TPC]
        return np.ascontiguousarray(
            a[:, c * TPC : (c + 1) * TPC].reshape(D // 128, 128, TPC).transpose(1, 0, 2)
        )

    in1 = [
        {
            "xhi": _ptile(xT, c),
            "xlo": _ptile(xTlo, c),
            "wg1hi": wg1hi,
            "wg1lo": wg1lo,
            "wg2": wg2p,
        }
        for c in core_ids
    ]
    res1 = run_bass_kernel_spmd(_CACHE["gate"], in1, core_ids).results
    # probs[p, i, e] -> token c*TPC + i*128 + p
    probs = np.concatenate(
        [res1[c]["probs"].transpose(1, 0, 2).reshape(TPC, E) for c in core_ids], axis=0
    ).astype(np.float32)  # [N, E]

    # ---------------- host routing (comparisons/indexing only) ----------------
    top2 = np.argsort(-probs, axis=1, kind="stable")[:, :2]  # [N, 2]
    e0s, e1s = top2[:, 0], top2[:, 1]
    expert_lists = [np.nonzero((top2 == e).any(axis=1))[0] for e in range(E)]
    counts = np.array([len(t) for t in expert_lists])

    tpl, asg = _solve_packing(counts)
    C = sum(tpl)

    # slot positions per size: (core, slot_idx) pools
    pos_pool = {}
    for jj, s in enumerate(tpl):
        pos_pool.setdefault(s, []).extend((c, jj) for c in core_ids)
    slot_off = {}  # (core, j) -> global row offset in yall
    for c in core_ids:
        off = c * C
        for jj, s in enumerate(tpl):
            slot_off[(c, jj)] = off
            off += s
    # assign slots to experts, chop token lists
    slot_map = {}  # (core, j) -> (expert, token_array)
    glob_row = np.zeros((N, E), np.int64)
    for e in range(E):
        tl = expert_lists[e]
        claims = []
        for s in sorted(asg.get(e, {}), reverse=True):
            for _ in range(asg[e][s]):
                claims.append((s, pos_pool[s].pop()))
        cum = 0
        for s, posn in claims:
            chunk = tl[cum : cum + s]
            slot_map[posn] = (e, chunk)
            glob_row[chunk, e] = slot_off[posn] + np.arange(len(chunk))
            cum += s
        assert cum >= len(tl), f"expert {e} not fully covered"
    # leftover slots empty
    for s, pool in pos_pool.items():
        for posn in pool:
            slot_map[posn] = (0, np.zeros(0, np.int64))

    w_tok = np.take_along_axis(probs, top2, axis=1)  # [N, 2] combine weights

    # ---------------- mlp: fused expert MLP on device ----------------
    key2 = (("emlp8" if USE_FP8 else "emlp"), tpl)
    if key2 not in _CACHE:
        _CACHE[key2] = _build_mlp_fp8_nc(tpl) if USE_FP8 else _build_mlp_nc(tpl)
    Wb = {}

    def _mtile(a):
        # [128, K/128, M] -> [M/128, 128, K/128, 128] (m-tile contiguous)
        P, KT, M = a.shape
        return np.ascontiguousarray(
            a.reshape(P, KT, M // 128, 128).transpose(2, 0, 1, 3)
        )

    def _split8(W, sw):
        Ws = W * sw
        hi = Ws.astype(E4M3)
        lo = (Ws - hi.astype(np.float32)).astype(E4M3)
        # concat hi and lo along the k-tile axis: [MT, 128, 2*KT, 128]
        return np.ascontiguousarray(
            np.concatenate([_mtile(_pmn(hi, E4M3)), _mtile(_pmn(lo, E4M3))], axis=2)
        )

    def wts(e):
        if e not in Wb:
            if USE_FP8:
                Wb[e] = (
                    _split8(W1[e], SW1),
                    _split8(W2[e], SW2),
                    _split8(W3[e], SW3),
                )
            else:
                Wb[e] = (_mtile(_pmn(W1[e])), _mtile(_pmn(W2[e])), _mtile(_pmn(W3[e])))
        return Wb[e]

    if USE_FP8:
        Xs = xT32 * SX
        xhi8 = Xs.astype(E4M3)
        xlo8 = (Xs - xhi8.astype(np.float32)).astype(E4M3)

    wmask = np.zeros((N, E), np.float32)
    wmask[np.arange(N), e0s] = w_tok[:, 0]
    wmask[np.arange(N), e1s] = w_tok[:, 1]

    def _pt(a, S):  # [D, S] -> [128, D/128, S]
        return np.ascontiguousarray(a.reshape(D // 128, 128, S).transpose(1, 0, 2))

    in2 = []
    for c in core_ids:
        d = {}
        for jj, S in enumerate(tpl):
            e, chunk = slot_map[(c, jj)]
            padded = np.zeros(S, np.int64)
            padded[: len(chunk)] = chunk
            wv = np.zeros(S, np.float32)
            wv[: len(chunk)] = wmask[chunk, e]
            if USE_FP8:
                d[f"xhi{jj}"] = _pt(xhi8[:, padded], S)
                d[f"xlo{jj}"] = _pt(xlo8[:, padded], S)
                d[f"w1_{jj}"], d[f"w2_{jj}"], d[f"w3_{jj}"] = wts(e)
                wv = wv / (SW3 * SH)
            else:
                w1p, w2p, w3p = wts(e)
                d[f"xt{jj}"] = _pt(xT[:, padded], S)
                d[f"w1_{jj}"] = w1p
                d[f"w2_{jj}"] = w2p
                d[f"w3_{jj}"] = w3p
            d[f"wv{jj}"] = np.ascontiguousarray(
                np.broadcast_to(wv[None, :], (128, S))
            )
        in2.append(d)
    res2 = run_bass_kernel_spmd(_CACHE[key2], in2, core_ids).results

    R = NCORES * C
    yall = np.empty((R, O), BF)
    for c in core_ids:
        for jj, S in enumerate(tpl):
            yt = res2[c][f"yt{jj}"]  # [128, O/128, S]
            off = slot_off[(c, jj)]
            yall[off : off + S] = yt.transpose(2, 1, 0).reshape(S, O)

    # ---------------- combine: gather 2 scaled rows + add ----------------
    key3 = ("comb", R)
    if key3 not in _CACHE:
        _CACHE[key3] = _build_comb_nc(R)
    g0 = glob_row[np.arange(N), e0s].astype(np.int32)
    g1 = glob_row[np.arange(N), e1s].astype(np.int32)
    in3 = []
    for c in core_ids:
        sl = slice(c * TPC, (c + 1) * TPC)
        in3.append(
            {
                "yall": yall,
                "i0": np.ascontiguousarray(g0[sl].reshape(NT, 128).T),
                "i1": np.ascontiguousarray(g1[sl].reshape(NT, 128).T),
            }
        )
    res3 = run_bass_kernel_spmd(_CACHE[key3], in3, core_ids).results
    out = np.concatenate(
        [
            res3[c]["out"].transpose(1, 0, 2).reshape(TPC, O).astype(np.float32)
            for c in core_ids
        ],
        axis=0,
    )
    return out
